# revision 28
# baseline (speedup 1.0000x reference)
"""Trainium2 Bass kernel for an attention-augmented LSTM (CaptioningRNN).

Reference computation (per batch n, T timesteps):
    A_flat = A.reshape(N, H, 16); h0 = c0 = A_flat.mean(-1)
    scores_t = (h_{t-1} @ A_flat) / sqrt(H); w = softmax(scores)
    attn_t = A_flat @ w
    a = x_t @ Wx + h_{t-1} @ Wh + attn_t @ Wattn + b
    i, f, o, g = split(a, 4); c_t = sig(f)*c + sig(i)*tanh(g); h_t = sig(o)*tanh(c_t)

Strategy: data-parallel over batch across 8 cores (32 batch rows each).
Per core:
  Phase A: U = x @ Wx + b precomputed for all timesteps (bf16 weights,
           rows t-major) and staged to DRAM in bf16. W2/AT/h0 for phase B
           are DMA'd concurrently on the gpsimd queue.
  Phase B: 64 recurrent steps. Gate matmul = [h; attn] (2048-dim contraction,
           bf16) against W2 = [Wh; Wattn] with gate-interleaved columns so each
           512-column block yields a full 128-dim slice of (i,f,o,g) and thus a
           128-dim slice of h/c. Attention scores are computed on the tensor
           engine (hT^T @ AT giving all batch pairs, diagonal extracted via a
           mask + strided reduce), softmax on ACT/DVE, attention pooling on
           DVE, h transposed back to hT layout with PE transposes into a
           shared PSUM bank (scores live in spare partitions 32..63 of the
           block-0 gate bank). W2/AT preload overlaps phase A's compute.

Weight-matrix column order (gate interleave): block j (512 cols) holds
original columns [i|f|o|g][j*128:(j+1)*128]. The same permutation is applied
to Wx, b and hence U.
"""

import math
import os

import numpy as np
import ml_dtypes

import concourse.bass as bass
import concourse.mybir as mybir
import concourse.tile as tile
from concourse import bacc

N, T, D, H = 256, 64, 1024, 1024
NCORES = 8
NB = N // NCORES          # 32 batch rows per core
G = 4 * H                 # 4096 gate columns
P = 16                    # attention positions (4x4)
KH = H // 128             # 8 contraction chunks for h
K2 = (2 * H) // 128       # 16 contraction chunks for [h; attn]
GB = G // 512             # 8 gate blocks of 512
F32 = mybir.dt.float32
BF16 = mybir.dt.bfloat16
BF = ml_dtypes.bfloat16

AF = mybir.ActivationFunctionType
ALU = mybir.AluOpType
AXX = mybir.AxisListType.X

_NC_CACHE = {}


def _gate_perm():
    """perm[new_col] = old_col for the gate-interleaved layout."""
    perm = np.empty(G, dtype=np.int64)
    for j in range(GB):
        for s in range(4):  # i, f, o, g
            perm[j * 512 + s * 128:(j * 512 + (s + 1) * 128)] = np.arange(
                s * H + j * 128, s * H + (j + 1) * 128)
    return perm


def build_nc(t_steps=T):
    """Build the SPMD Bass program (identical on all cores)."""
    nc = bacc.Bacc("TRN2", target_bir_lowering=False, debug=False,
                   num_devices=NCORES)

    xT_d = nc.dram_tensor("xT", [D, t_steps * NB], BF16, kind="ExternalInput")
    wx_d = nc.dram_tensor("wx", [D, G], BF16, kind="ExternalInput")
    w2_d = nc.dram_tensor("w2", [2 * H, G], BF16, kind="ExternalInput")
    b128_d = nc.dram_tensor("b128", [128, G], F32, kind="ExternalInput")
    at_d = nc.dram_tensor("at", [H, NB * P], BF16, kind="ExternalInput")
    h0Tq_d = nc.dram_tensor("h0Tq", [2 * 128, 128], BF16, kind="ExternalInput")
    h0q_d = nc.dram_tensor("h0q", [2 * 128, 128], F32, kind="ExternalInput")
    mask_d = nc.dram_tensor("mask", [NB, NB * P], BF16, kind="ExternalInput")
    ones_d = nc.dram_tensor("ones", [1, 128], BF16, kind="ExternalInput")
    ident_d = nc.dram_tensor("ident", [NB, NB], BF16, kind="ExternalInput")
    eye128_d = nc.dram_tensor("eye128", [128, 128], BF16,
                              kind="ExternalInput")
    out_d = nc.dram_tensor("out", [NB, t_steps, H], F32, kind="ExternalOutput")

    n_row_tiles = (t_steps * NB) // 128

    with tile.TileContext(nc) as tc:
        with tc.tile_pool(name="dram", bufs=1, space="DRAM") as dpool, \
             tc.tile_pool(name="res", bufs=1) as res:
            u_dram = dpool.tile([t_steps * NB, G], BF16)

            # phase-B resident tiles; DMAs issued inside phase A below so
            # the xT loads (needed first) win the HBM bandwidth race
            w2 = [res.tile([128, G], BF16, tag=f"w2_{k}", name=f"w2_{k}")
                  for k in range(K2)]
            at_all = res.tile([128, KH * NB * P], BF16, tag="at_all")
            at = [at_all[:, k * NB * P:(k + 1) * NB * P]
                  for k in range(KH)]
            mask = res.tile([NB, NB * P], BF16, tag="mask")
            ones = res.tile([1, 128], BF16, tag="ones")
            ident = res.tile([NB, NB], BF16, tag="ident")
            eye128 = res.tile([128, 128], BF16, tag="eye128")

            # ---------------- Phase A: U = x @ Wx + b ----------------
            # g-outer so only a 512-col slice of Wx/b is resident, leaving
            # room for the W2 preload above.
            with tc.tile_pool(name="pa", bufs=1) as pa, \
                 tc.tile_pool(name="pa_ps", bufs=8, space="PSUM") as pa_ps, \
                 tc.tile_pool(name="pa_sb", bufs=12) as pa_sb:
                xT = []
                qs = [nc.sync, nc.scalar, nc.gpsimd]
                for d in range(KH):
                    t_ = pa.tile([128, t_steps * NB], BF16, tag=f"xT{d}")
                    qs[d % 3].dma_start(t_[:], xT_d[d * 128:(d + 1) * 128, :])
                    xT.append(t_)
                # small phase-B residents next, then the big W2 preload last
                for k in range(KH):
                    nc.gpsimd.dma_start(
                        at_all[:, k * NB * P:(k + 1) * NB * P],
                        at_d[k * 128:(k + 1) * 128, :])
                nc.gpsimd.dma_start(mask[:], mask_d[:])
                nc.gpsimd.dma_start(ones[:], ones_d[:])
                nc.gpsimd.dma_start(ident[:], ident_d[:])
                nc.gpsimd.dma_start(eye128[:], eye128_d[:])
                for k in range(K2):
                    nc.gpsimd.dma_start(w2[k][:], w2_d[k * 128:(k + 1) * 128, :])

                for g in range(GB):
                    gs = slice(g * 512, (g + 1) * 512)
                    wxg = pa.tile([128, KH * 512], BF16, tag="wxg", bufs=2)
                    for d in range(KH):
                        nc.scalar.dma_start(
                            wxg[:, d * 512:(d + 1) * 512],
                            wx_d[d * 128:(d + 1) * 128, gs])
                    bg = pa.tile([128, 512], F32, tag="bg", bufs=2)
                    nc.scalar.dma_start(bg[:], b128_d[:, gs])
                    for m in range(n_row_tiles):
                        ms = slice(m * 128, (m + 1) * 128)
                        ps = pa_ps.tile([128, 512], F32, tag="ps")
                        for d in range(KH):
                            nc.tensor.matmul(
                                ps[:], xT[d][:, ms],
                                wxg[:, d * 512:(d + 1) * 512],
                                start=(d == 0), stop=(d == KH - 1))
                        us = pa_sb.tile([128, 512], BF16, tag="us")
                        nc.vector.tensor_add(us[:], ps[:], bg[:])
                        nc.sync.dma_start(u_dram[ms, gs], us[:])

            # ---------------- Phase B: recurrence ----------------
            with tc.tile_pool(name="ht", bufs=4) as htp, \
                 tc.tile_pool(name="u", bufs=2) as up, \
                 tc.tile_pool(name="st", bufs=2) as stp, \
                 tc.tile_pool(name="att", bufs=3) as attp, \
                 tc.tile_pool(name="abt", bufs=10) as abtp, \
                 tc.tile_pool(name="blk", bufs=2) as blkp, \
                 tc.tile_pool(name="psg", bufs=6, space="PSUM") as psg_p, \
                 tc.tile_pool(name="pst", bufs=1, space="PSUM") as pst_p, \
                 tc.tile_pool(name="psw", bufs=1, space="PSUM") as psw_p:

                hbfT = []
                for q in range(2):
                    t_ = htp.tile([128, 128], BF16, tag="hbfT",
                                  name=f"h0T_{q}")
                    nc.gpsimd.dma_start(t_[:],
                                        h0Tq_d[q * 128:(q + 1) * 128, :])
                    hbfT.append(t_)
                c_b = []
                for q in range(2):
                    t_ = blkp.tile([128, 128], F32, tag="c", bufs=4,
                                   name=f"c0_{q}")
                    nc.gpsimd.dma_start(t_[:], h0q_d[q * 128:(q + 1) * 128, :])
                    c_b.append(t_)

                def hsl(ht2, k):
                    # hT chunk k as a slice of the quad-transposed tiles
                    return ht2[k // 4][:, 32 * (k % 4):32 * (k % 4) + 32]

                u_t = up.tile([NB, G], BF16, tag="u")
                nc.gpsimd.dma_start(u_t[:], u_dram[0:NB, :])

                inv_sqrt_h = 1.0 / math.sqrt(H)

                def smm(pg, k, lhs, rhs, start, stop):
                    if isinstance(lhs, tile.Tile):
                        lhs = lhs[:]
                    j = k % 2
                    nc.tensor.matmul(pg[64 * j:64 * j + NB, :], lhs, rhs,
                                     start=start, stop=stop,
                                     tile_position=(0, 64 * j),
                                     skip_group_check=True)

                def umm(pg, gsl, u):
                    nc.tensor.matmul(pg[0:NB, :], ident[:], u[:, gsl],
                                     start=False, stop=False,
                                     tile_position=(0, 0),
                                     skip_group_check=True)

                # ---- prologue: scores S_0 (in spare partitions 96..127
                # of block 0's PSUM bank) + h-part of blocks 0..3 + U ----
                psg = {}
                for g in range(4):
                    gsl = slice(g * 512, (g + 1) * 512)
                    pg = psg_p.tile([128, 512], F32, tag="g", name=f"pg{g}")
                    if g == 0:
                        ps_s = pg[32:64, :]
                        for k in range(KH):
                            nc.tensor.matmul(ps_s, hsl(hbfT, k), at[k],
                                             start=(k == 0),
                                             stop=(k == KH - 1),
                                             skip_group_check=True)
                    for k in range(KH):
                        smm(pg, k, hsl(hbfT, k), w2[k][:, gsl], k < 2, False)
                    umm(pg, gsl, u_t)
                    psg[g] = pg

                for t in range(t_steps):
                    last = (t + 1 >= t_steps)
                    if not last:
                        u_next = up.tile([NB, G], BF16, tag="u")
                        nc.gpsimd.dma_start(
                            u_next[:], u_dram[(t + 1) * NB:(t + 2) * NB, :])

                    # (a) softmax chain for step t (scores psum -> w1)
                    sm_sc = nc.enter_named_scope(f"sm{t}", False)
                    masked = stp.tile([NB, NB * P], F32, tag="masked")
                    nc.vector.tensor_tensor(
                        out=masked[:].rearrange("m (p n) -> m p n", n=NB),
                        in0=ps_s.rearrange("m (n p) -> m p n", p=P),
                        in1=mask[:].rearrange("m (n p) -> m p n", p=P),
                        op=ALU.mult)
                    sc = stp.tile([NB, P], F32, tag="sc")
                    nc.vector.tensor_reduce(
                        sc[:], masked[:].rearrange("m (p n) -> m p n", n=NB),
                        axis=AXX, op=ALU.add)
                    # exp(x) = s/(1-s) with s = sigmoid(x): keeps the ACT
                    # table cache at {Sigmoid, Tanh} with no per-step reloads
                    sg = stp.tile([NB, P], F32, tag="sg")
                    nc.scalar.activation(sg[:], sc[:], AF.Sigmoid,
                                         scale=float(inv_sqrt_h))
                    om = stp.tile([NB, P], F32, tag="om")
                    nc.scalar.activation(om[:], sc[:], AF.Sigmoid,
                                         scale=float(-inv_sqrt_h))
                    omr = stp.tile([NB, P], F32, tag="omr")
                    nc.vector.reciprocal(omr[:], om[:])
                    expw = stp.tile([NB, P], F32, tag="expw")
                    nc.vector.tensor_tensor(out=expw[:], in0=sg[:], in1=omr[:],
                                            op=ALU.mult)
                    sume = stp.tile([NB, 1], F32, tag="sume")
                    nc.vector.tensor_reduce(sume[:], expw[:], axis=AXX,
                                            op=ALU.add)
                    rec = stp.tile([NB, 1], F32, tag="rec")
                    nc.vector.reciprocal(rec[:], sume[:])
                    w16 = stp.tile([NB, P], BF16, tag="w16")
                    nc.vector.tensor_scalar(out=w16[:], in0=expw[:],
                                            scalar1=rec[:], scalar2=None,
                                            op0=ALU.mult)
                    # flatten [NB, P] -> [1, NB*P]: direct SBUF->SBUF gather
                    w1 = stp.tile([1, NB * P], BF16, tag="w1")
                    nc.sync.dma_start(w1[:], w16[:])
                    nc.leave_named_scope(f"sm{t}", sm_sc[0], False)

                    # (d4) block 4: h-part + U (covers the softmax latency)
                    sc_ = nc.enter_named_scope(f"d4_{t}", False)
                    for g in (4,):
                        gsl = slice(g * 512, (g + 1) * 512)
                        pg = psg_p.tile([128, 512], F32, tag="g",
                                        name=f"pg4_{g}")
                        for k in range(KH):
                            smm(pg, k, hsl(hbfT, k), w2[k][:, gsl], k < 2,
                                False)
                        umm(pg, gsl, u_t)
                        psg[g] = pg
                    nc.leave_named_scope(f"d4_{t}", sc_[0], False)

                    # (d5) block 5: h-part + U (covers the softmax too)
                    sc_ = nc.enter_named_scope(f"att{t}", False)
                    for g in (5,):
                        gsl = slice(g * 512, (g + 1) * 512)
                        pg = psg_p.tile([128, 512], F32, tag="g",
                                        name=f"pg5_{g}")
                        for k in range(KH):
                            smm(pg, k, hsl(hbfT, k), w2[k][:, gsl], k < 2,
                                False)
                        umm(pg, gsl, u_t)
                        psg[g] = pg

                    # (b) broadcast w to 128 partitions via ones-matmul;
                    # psum->sbuf bf16 copy on ACT (keeps DVE free)
                    ps_w = psw_p.tile([128, NB * P], F32, tag="w")
                    nc.tensor.matmul(ps_w[:], ones[:], w1[:],
                                     start=True, stop=True)
                    wfull = attp.tile([128, NB * P], BF16, tag="wfull")
                    nc.scalar.activation(wfull[:], ps_w[:], AF.Copy)

                    # (e) attention pooling -> attnT: product on DVE/GpSimd
                    # (split), reduce on DVE, over the packed AT tile
                    attnT = []
                    with nc.allow_low_precision("attn pooled in bf16 anyway"):
                        # chunk 0 alone first (earliest consumer), then pairs
                        groups = [(0, 1), (1, 3), (3, 5), (5, 7), (7, 8)]
                        for lo, hi in groups:
                            nk = hi - lo
                            hs = slice(lo * NB * P, hi * NB * P)
                            pr = attp.tile([128, 2 * NB * P], BF16, tag="pr")
                            prv = pr[:, 0:nk * NB * P]
                            peng = nc.vector if lo < 3 else nc.gpsimd
                            peng.tensor_tensor(
                                out=prv.rearrange("m (k x) -> m k x", k=nk),
                                in0=at_all[:, hs].rearrange(
                                    "m (k x) -> m k x", k=nk),
                                in1=bass.AP(wfull[:].tensor, wfull[:].offset,
                                            [wfull[:].ap[0], [0, nk],
                                             wfull[:].ap[1]]),
                                op=ALU.mult)
                            ab4 = abtp.tile([128, 2 * NB], BF16, tag="ab")
                            nc.vector.tensor_reduce(
                                ab4[:, 0:nk * NB],
                                prv.rearrange("m (k n p) -> m k n p", k=nk,
                                              p=P),
                                axis=AXX, op=ALU.add)
                            for kk in range(nk):
                                attnT.append(
                                    ab4[:, kk * NB:(kk + 1) * NB])
                    nc.leave_named_scope(f"att{t}", sc_[0], False)

                    # quad-stacked state for this step: blocks 4g'..4g'+3 of
                    # quad q live on partitions 32g'..32g'+31
                    GS = [blkp.tile([128, 512], F32, tag=f"GS{q}",
                                    name=f"GS{q}_{t}") for q in range(2)]
                    c_new = [blkp.tile([128, 128], F32, tag="c", bufs=4,
                                       name=f"cn{q}_{t}") for q in range(2)]
                    hbf = [blkp.tile([128, 128], BF16, tag="hbf",
                                     name=f"hbf{q}_{t}") for q in range(2)]
                    if not last:
                        hbfT_new = [htp.tile([128, 128], BF16, tag="hbfT",
                                             name=f"hT{q}_{t}")
                                    for q in range(2)]

                    def merge_block(g, pg):
                        # strip1 + strip0(+U) -> stacked row of GS[quad]
                        q, gp = divmod(g, 4)
                        row = slice(32 * gp, 32 * gp + 32)
                        g1 = blkp.tile([NB, 512], F32, tag="g1", bufs=3)
                        nc.scalar.activation(g1[:], pg[64:64 + NB, :], AF.Copy)
                        nc.vector.tensor_add(GS[q][row, :], pg[0:NB, :],
                                             g1[:])

                    def quad_math(q):
                        # all four blocks of the quad at full partition width
                        gq = GS[q]
                        sio = blkp.tile([128, 384], F32, tag="sio")
                        nc.scalar.activation(sio[:], gq[:, 0:384], AF.Sigmoid)
                        tg = blkp.tile([128, 128], F32, tag="tg")
                        nc.scalar.activation(tg[:], gq[:, 384:512], AF.Tanh)
                        m1 = blkp.tile([128, 128], F32, tag="m1")
                        nc.vector.tensor_tensor(out=m1[:], in0=sio[:, 0:128],
                                                in1=tg[:], op=ALU.mult)
                        m2 = blkp.tile([128, 128], F32, tag="m2")
                        nc.vector.tensor_tensor(out=m2[:], in0=sio[:, 128:256],
                                                in1=c_b[q][:], op=ALU.mult)
                        nc.vector.tensor_add(c_new[q][:], m1[:], m2[:])
                        tcn = blkp.tile([128, 128], F32, tag="tcn")
                        nc.scalar.activation(tcn[:], c_new[q][:], AF.Tanh)
                        nc.vector.tensor_tensor(out=hbf[q][:],
                                                in0=sio[:, 256:384],
                                                in1=tcn[:], op=ALU.mult)
                        # fp32 copy for the DRAM output
                        hf = blkp.tile([128, 128], F32, tag="hf", bufs=4,
                                       name=f"hf{q}_{t}")
                        nc.scalar.activation(hf[:], hbf[q][:], AF.Copy)
                        qsl = slice(q * 512, (q + 1) * 512)
                        nc.sync.dma_start(
                            out_d[:, t, qsl].rearrange("n (g c) -> g n c",
                                                       g=4),
                            hf[:])

                    # (f) attn-parts for blocks 0..5: first chunks k-outer so
                    # the matmuls pipeline against attnT production, then
                    # per-block finish; quad0 math after block 3's merge
                    sc_ = nc.enter_named_scope(f"f05_{t}", False)
                    for k in range(KH, KH + 4):
                        for g in range(6):
                            gsl = slice(g * 512, (g + 1) * 512)
                            smm(psg[g], k, attnT[k - KH], w2[k][:, gsl],
                                False, False)
                    for g in range(6):
                        gsl = slice(g * 512, (g + 1) * 512)
                        for k in range(KH + 4, K2):
                            smm(psg[g], k, attnT[k - KH], w2[k][:, gsl],
                                False, k >= K2 - 2)
                        merge_block(g, psg[g])
                        if g == 3:
                            quad_math(0)
                    nc.leave_named_scope(f"f05_{t}", sc_[0], False)

                    # (h) blocks 6,7: full accumulation + merges + quad1
                    sc_ = nc.enter_named_scope(f"h67_{t}", False)
                    for g in (6, 7):
                        gsl = slice(g * 512, (g + 1) * 512)
                        pg = psg_p.tile([128, 512], F32, tag="g",
                                        name=f"pg67_{g}")
                        for k in range(KH):
                            smm(pg, k, hsl(hbfT, k), w2[k][:, gsl], k < 2,
                                False)
                        umm(pg, gsl, u_t)
                        for k in range(KH, K2):
                            smm(pg, k, attnT[k - KH], w2[k][:, gsl],
                                False, k >= K2 - 2)
                        merge_block(g, pg)
                    quad_math(1)
                    nc.leave_named_scope(f"h67_{t}", sc_[0], False)

                    # (i..l) PE-transpose h back to hT layout, next
                    # step's scores first (they gate the softmax), then
                    # blocks 0..3 h-part + U
                    sc_ = nc.enter_named_scope(f"nxt{t}", False)
                    if not last:
                        psg2 = {}
                        for g in range(4):
                            psg2[g] = psg_p.tile([128, 512], F32, tag="g",
                                                 name=f"pgn{g}")
                        ps_s = psg2[0][32:64, :]
                        # PE transposes -> one shared psum bank -> sbuf
                        htq = pst_p.tile([128, 256], BF16, tag="htq",
                                         name=f"htq_{t}")
                        nc.tensor.transpose(htq[:, 0:128], hbf[0][:],
                                            eye128[:])
                        nc.vector.tensor_copy(hbfT_new[0][:], htq[:, 0:128])
                        for k in range(4):
                            nc.tensor.matmul(ps_s, hsl(hbfT_new, k), at[k],
                                             start=(k == 0), stop=False,
                                             skip_group_check=True)
                        # quad1 transpose
                        nc.tensor.transpose(htq[:, 128:256], hbf[1][:],
                                            eye128[:])
                        nc.vector.tensor_copy(hbfT_new[1][:], htq[:, 128:256])
                        for k in (4, 5, 6, 7):
                            nc.tensor.matmul(ps_s, hsl(hbfT_new, k), at[k],
                                             start=False, stop=(k == 7),
                                             skip_group_check=True)
                        for g in range(4):
                            gsl = slice(g * 512, (g + 1) * 512)
                            for k in range(KH):
                                smm(psg2[g], k, hsl(hbfT_new, k),
                                    w2[k][:, gsl], k < 2, False)
                            umm(psg2[g], gsl, u_next)
                        psg = psg2
                    nc.leave_named_scope(f"nxt{t}", sc_[0], False)

                    if not last:
                        hbfT = hbfT_new
                        c_b = c_new
                        u_t = u_next

    nc.compile()
    return nc


def prepare_inputs(x, A, Wx, Wh, Wattn, b, t_steps=T):
    """Host-side sharding + layout prep. Returns list of per-core input maps."""
    x = np.asarray(x, dtype=np.float32)
    A = np.asarray(A, dtype=np.float32)
    Wx = np.asarray(Wx, dtype=np.float32)
    Wh = np.asarray(Wh, dtype=np.float32)
    Wattn = np.asarray(Wattn, dtype=np.float32)
    b = np.asarray(b, dtype=np.float32)

    perm = _gate_perm()
    wx_p = np.ascontiguousarray(Wx[:, perm]).astype(BF)
    w2_p = np.ascontiguousarray(np.vstack([Wh, Wattn])[:, perm]).astype(BF)
    b128 = np.ascontiguousarray(
        np.broadcast_to(b[perm], (128, G))).astype(np.float32)
    mask = np.zeros((NB, NB * P), dtype=BF)
    for n in range(NB):
        mask[n, n * P:(n + 1) * P] = 1
    ones = np.ones((1, 128), dtype=BF)
    ident = np.eye(NB, dtype=BF)
    eye128 = np.eye(128, dtype=BF)

    in_maps = []
    for c in range(NCORES):
        x_c = x[c * NB:(c + 1) * NB, :t_steps]          # (NB, t, D)
        xr = x_c.transpose(1, 0, 2).reshape(t_steps * NB, D)  # t-major rows
        xT = np.ascontiguousarray(xr.T).astype(BF)       # (D, t*NB)
        A_c = A[c * NB:(c + 1) * NB].reshape(NB, H, P)
        at_c = np.ascontiguousarray(
            A_c.transpose(1, 0, 2).reshape(H, NB * P)).astype(BF)
        h0 = A_c.mean(axis=2).astype(np.float32)         # (NB, H)
        # quad-transposed initial h: tile q, col 32*g'+n, row c ->
        # h[n, (4q+g')*128 + c]
        h0Tq = np.empty((2 * 128, 128), dtype=BF)
        for k in range(8):
            q, gp = divmod(k, 4)
            h0Tq[q * 128:(q + 1) * 128, gp * 32:(gp + 1) * 32] = \
                h0[:, k * 128:(k + 1) * 128].T.astype(BF)
        # quad-stacked initial cell state: block g of quad q lives on
        # partitions 32*(g%4), columns = h dims within the block
        h0q = np.empty((2 * 128, 128), dtype=np.float32)
        for g in range(8):
            q, gp = divmod(g, 4)
            h0q[q * 128 + gp * 32:q * 128 + (gp + 1) * 32, :] = \
                h0[:, g * 128:(g + 1) * 128]
        in_maps.append({
            "xT": xT, "wx": wx_p, "w2": w2_p, "b128": b128,
            "at": at_c, "h0Tq": h0Tq, "h0q": h0q,
            "mask": mask, "ones": ones, "ident": ident,
            "eye128": eye128,
        })
    return in_maps


def kernel(x, A, Wx, Wh, Wattn, b):
    from concourse.bass_utils import run_bass_kernel_spmd

    key = T
    if key not in _NC_CACHE:
        _NC_CACHE[key] = build_nc(T)
    nc = _NC_CACHE[key]

    in_maps = prepare_inputs(x, A, Wx, Wh, Wattn, b)
    trace = bool(int(os.environ.get("KERNEL_TRACE", "0")))
    res = run_bass_kernel_spmd(nc, in_maps, core_ids=list(range(NCORES)),
                               trace=trace)
    if res.exec_time_ns is not None:
        print(f"HW exec time: {res.exec_time_ns} ns")
        kernel.last_exec_time_ns = res.exec_time_ns
    out = np.concatenate([r["out"] for r in res.results], axis=0)
    return out.astype(np.float32)


kernel.last_exec_time_ns = None


# revision 30
# speedup vs baseline: 1.0464x; 1.0464x over previous
"""Trainium2 Bass kernel for an attention-augmented LSTM (CaptioningRNN).

Reference computation (per batch n, T timesteps):
    A_flat = A.reshape(N, H, 16); h0 = c0 = A_flat.mean(-1)
    scores_t = (h_{t-1} @ A_flat) / sqrt(H); w = softmax(scores)
    attn_t = A_flat @ w
    a = x_t @ Wx + h_{t-1} @ Wh + attn_t @ Wattn + b
    i, f, o, g = split(a, 4); c_t = sig(f)*c + sig(i)*tanh(g); h_t = sig(o)*tanh(c_t)

Strategy: data-parallel over batch across 8 cores (32 batch rows each).
Per core:
  Phase A: U = x @ Wx + b precomputed for all timesteps (bf16 weights,
           rows t-major) and staged to DRAM in bf16. W2/AT/h0 for phase B
           are DMA'd concurrently on the gpsimd queue.
  Phase B: 64 recurrent steps. Gate matmul = [h; attn] (2048-dim contraction,
           bf16) against W2 = [Wh; Wattn] with gate-interleaved columns so each
           512-column block yields a full 128-dim slice of (i,f,o,g) and thus a
           128-dim slice of h/c. Attention scores are computed on the tensor
           engine (hT^T @ AT giving all batch pairs, diagonal extracted via a
           mask + strided reduce), softmax on ACT/DVE, attention pooling on
           DVE, h transposed back to hT layout with PE transposes into a
           shared PSUM bank (scores live in spare partitions 32..63 of the
           block-0 gate bank). W2/AT preload overlaps phase A's compute.

Weight-matrix column order (gate interleave): block j (512 cols) holds
original columns [i|f|o|g][j*128:(j+1)*128]. The same permutation is applied
to Wx, b and hence U.
"""

import math
import os

import numpy as np
import ml_dtypes

import concourse.bass as bass
import concourse.mybir as mybir
import concourse.tile as tile
from concourse import bacc

N, T, D, H = 256, 64, 1024, 1024
NCORES = 8
NB = N // NCORES          # 32 batch rows per core
G = 4 * H                 # 4096 gate columns
P = 16                    # attention positions (4x4)
KH = H // 128             # 8 contraction chunks for h
K2 = (2 * H) // 128       # 16 contraction chunks for [h; attn]
GB = G // 512             # 8 gate blocks of 512
F32 = mybir.dt.float32
BF16 = mybir.dt.bfloat16
BF = ml_dtypes.bfloat16

AF = mybir.ActivationFunctionType
ALU = mybir.AluOpType
AXX = mybir.AxisListType.X

_NC_CACHE = {}


def _gate_perm():
    """perm[new_col] = old_col for the gate-interleaved layout."""
    perm = np.empty(G, dtype=np.int64)
    for j in range(GB):
        for s in range(4):  # i, f, o, g
            perm[j * 512 + s * 128:(j * 512 + (s + 1) * 128)] = np.arange(
                s * H + j * 128, s * H + (j + 1) * 128)
    return perm


def build_nc(t_steps=T):
    """Build the SPMD Bass program (identical on all cores)."""
    nc = bacc.Bacc("TRN2", target_bir_lowering=False, debug=False,
                   num_devices=NCORES)

    xT_d = nc.dram_tensor("xT", [D, t_steps * NB], BF16, kind="ExternalInput")
    wx_d = nc.dram_tensor("wx", [D, G], BF16, kind="ExternalInput")
    w2_d = nc.dram_tensor("w2", [2 * H, G], BF16, kind="ExternalInput")
    b128_d = nc.dram_tensor("b128", [128, G], F32, kind="ExternalInput")
    at_d = nc.dram_tensor("at", [H, NB * P], BF16, kind="ExternalInput")
    h0Tq_d = nc.dram_tensor("h0Tq", [2 * 128, 128], BF16, kind="ExternalInput")
    h0q_d = nc.dram_tensor("h0q", [2 * 128, 128], F32, kind="ExternalInput")
    mask_d = nc.dram_tensor("mask", [NB, NB * P], BF16, kind="ExternalInput")
    ones_d = nc.dram_tensor("ones", [1, 128], BF16, kind="ExternalInput")
    ident_d = nc.dram_tensor("ident", [NB, NB], BF16, kind="ExternalInput")
    eye128_d = nc.dram_tensor("eye128", [128, 128], BF16,
                              kind="ExternalInput")
    out_d = nc.dram_tensor("out", [NB, t_steps, H], F32, kind="ExternalOutput")

    n_row_tiles = (t_steps * NB) // 128

    with tile.TileContext(nc) as tc:
        with tc.tile_pool(name="dram", bufs=1, space="DRAM") as dpool, \
             tc.tile_pool(name="res", bufs=1) as res:
            u_dram = dpool.tile([t_steps * NB, G], BF16)

            # phase-B resident tiles; DMAs issued inside phase A below so
            # the xT loads (needed first) win the HBM bandwidth race
            w2 = [res.tile([128, G], BF16, tag=f"w2_{k}", name=f"w2_{k}")
                  for k in range(K2)]
            at_all = res.tile([128, KH * NB * P], BF16, tag="at_all")
            at = [at_all[:, k * NB * P:(k + 1) * NB * P]
                  for k in range(KH)]
            mask = res.tile([NB, NB * P], BF16, tag="mask")
            ones = res.tile([1, 128], BF16, tag="ones")
            ident = res.tile([NB, NB], BF16, tag="ident")
            eye128 = res.tile([128, 128], BF16, tag="eye128")

            # ---------------- Phase A: U = x @ Wx + b ----------------
            # g-outer so only a 512-col slice of Wx/b is resident, leaving
            # room for the W2 preload above.
            with tc.tile_pool(name="pa", bufs=1) as pa, \
                 tc.tile_pool(name="pa_ps", bufs=8, space="PSUM") as pa_ps, \
                 tc.tile_pool(name="pa_sb", bufs=12) as pa_sb:
                xT = []
                qs = [nc.sync, nc.scalar, nc.gpsimd]
                for d in range(KH):
                    t_ = pa.tile([128, t_steps * NB], BF16, tag=f"xT{d}")
                    qs[d % 3].dma_start(t_[:], xT_d[d * 128:(d + 1) * 128, :])
                    xT.append(t_)
                # small phase-B residents next, then the big W2 preload last
                for k in range(KH):
                    nc.gpsimd.dma_start(
                        at_all[:, k * NB * P:(k + 1) * NB * P],
                        at_d[k * 128:(k + 1) * 128, :])
                nc.gpsimd.dma_start(mask[:], mask_d[:])
                nc.gpsimd.dma_start(ones[:], ones_d[:])
                nc.gpsimd.dma_start(ident[:], ident_d[:])
                nc.gpsimd.dma_start(eye128[:], eye128_d[:])
                for k in range(K2):
                    nc.gpsimd.dma_start(w2[k][:], w2_d[k * 128:(k + 1) * 128, :])

                for g in range(GB):
                    gs = slice(g * 512, (g + 1) * 512)
                    wxg = pa.tile([128, KH * 512], BF16, tag="wxg", bufs=2)
                    for d in range(KH):
                        nc.scalar.dma_start(
                            wxg[:, d * 512:(d + 1) * 512],
                            wx_d[d * 128:(d + 1) * 128, gs])
                    bg = pa.tile([128, 512], F32, tag="bg", bufs=2)
                    nc.scalar.dma_start(bg[:], b128_d[:, gs])
                    for m in range(n_row_tiles):
                        ms = slice(m * 128, (m + 1) * 128)
                        ps = pa_ps.tile([128, 512], F32, tag="ps")
                        for d in range(KH):
                            nc.tensor.matmul(
                                ps[:], xT[d][:, ms],
                                wxg[:, d * 512:(d + 1) * 512],
                                start=(d == 0), stop=(d == KH - 1))
                        us = pa_sb.tile([128, 512], BF16, tag="us")
                        nc.vector.tensor_add(us[:], ps[:], bg[:])
                        nc.sync.dma_start(u_dram[ms, gs], us[:])

            # ---------------- Phase B: recurrence ----------------
            with tc.tile_pool(name="ht", bufs=6) as htp, \
                 tc.tile_pool(name="u", bufs=2) as up, \
                 tc.tile_pool(name="st", bufs=2) as stp, \
                 tc.tile_pool(name="att", bufs=4) as attp, \
                 tc.tile_pool(name="abt", bufs=12) as abtp, \
                 tc.tile_pool(name="blk", bufs=2) as blkp, \
                 tc.tile_pool(name="psg", bufs=6, space="PSUM") as psg_p, \
                 tc.tile_pool(name="pst", bufs=1, space="PSUM") as pst_p, \
                 tc.tile_pool(name="psw", bufs=1, space="PSUM") as psw_p:

                hbfT = []
                for q in range(2):
                    t_ = htp.tile([128, 128], BF16, tag="hbfT",
                                  name=f"h0T_{q}")
                    nc.gpsimd.dma_start(t_[:],
                                        h0Tq_d[q * 128:(q + 1) * 128, :])
                    hbfT.append(t_)
                c_b = []
                for q in range(2):
                    t_ = blkp.tile([128, 128], F32, tag="c", bufs=4,
                                   name=f"c0_{q}")
                    nc.gpsimd.dma_start(t_[:], h0q_d[q * 128:(q + 1) * 128, :])
                    c_b.append(t_)

                def hsl(ht2, k):
                    # hT chunk k as a slice of the quad-transposed tiles
                    return ht2[k // 4][:, 32 * (k % 4):32 * (k % 4) + 32]

                u_t = up.tile([NB, G], BF16, tag="u")
                nc.gpsimd.dma_start(u_t[:], u_dram[0:NB, :])

                inv_sqrt_h = 1.0 / math.sqrt(H)

                def smm(pg, k, lhs, rhs, start, stop):
                    if isinstance(lhs, tile.Tile):
                        lhs = lhs[:]
                    j = k % 2
                    nc.tensor.matmul(pg[64 * j:64 * j + NB, :], lhs, rhs,
                                     start=start, stop=stop,
                                     tile_position=(0, 64 * j),
                                     skip_group_check=True)

                def umm(pg, gsl, u):
                    nc.tensor.matmul(pg[0:NB, :], ident[:], u[:, gsl],
                                     start=False, stop=False,
                                     tile_position=(0, 0),
                                     skip_group_check=True)

                # ---- prologue: scores S_0 (in spare partitions 96..127
                # of block 0's PSUM bank) + h-part of blocks 0..3 + U ----
                psg = {}
                for g in range(4):
                    gsl = slice(g * 512, (g + 1) * 512)
                    pg = psg_p.tile([128, 512], F32, tag="g", name=f"pg{g}")
                    if g == 0:
                        ps_s = pg[32:64, :]
                        for k in range(KH):
                            nc.tensor.matmul(ps_s, hsl(hbfT, k), at[k],
                                             start=(k == 0),
                                             stop=(k == KH - 1),
                                             skip_group_check=True)
                    for k in range(KH):
                        smm(pg, k, hsl(hbfT, k), w2[k][:, gsl], k < 2, False)
                    umm(pg, gsl, u_t)
                    psg[g] = pg

                for t in range(t_steps):
                    last = (t + 1 >= t_steps)
                    if not last:
                        u_next = up.tile([NB, G], BF16, tag="u")
                        nc.gpsimd.dma_start(
                            u_next[:], u_dram[(t + 1) * NB:(t + 2) * NB, :])

                    # (a) softmax chain for step t (scores psum -> w1)
                    sm_sc = nc.enter_named_scope(f"sm{t}", False)
                    masked = stp.tile([NB, NB * P], F32, tag="masked")
                    nc.vector.tensor_tensor(
                        out=masked[:].rearrange("m (p n) -> m p n", n=NB),
                        in0=ps_s.rearrange("m (n p) -> m p n", p=P),
                        in1=mask[:].rearrange("m (n p) -> m p n", p=P),
                        op=ALU.mult)
                    sc = stp.tile([NB, P], F32, tag="sc")
                    nc.vector.tensor_reduce(
                        sc[:], masked[:].rearrange("m (p n) -> m p n", n=NB),
                        axis=AXX, op=ALU.add)
                    # exp(x) = s/(1-s) with s = sigmoid(x): keeps the ACT
                    # table cache at {Sigmoid, Tanh} with no per-step reloads
                    sg = stp.tile([NB, P], F32, tag="sg")
                    nc.scalar.activation(sg[:], sc[:], AF.Sigmoid,
                                         scale=float(inv_sqrt_h))
                    om = stp.tile([NB, P], F32, tag="om")
                    nc.scalar.activation(om[:], sc[:], AF.Sigmoid,
                                         scale=float(-inv_sqrt_h))
                    omr = stp.tile([NB, P], F32, tag="omr")
                    nc.vector.reciprocal(omr[:], om[:])
                    expw = stp.tile([NB, P], F32, tag="expw")
                    nc.vector.tensor_tensor(out=expw[:], in0=sg[:], in1=omr[:],
                                            op=ALU.mult)
                    sume = stp.tile([NB, 1], F32, tag="sume")
                    nc.vector.tensor_reduce(sume[:], expw[:], axis=AXX,
                                            op=ALU.add)
                    rec = stp.tile([NB, 1], F32, tag="rec")
                    nc.vector.reciprocal(rec[:], sume[:])
                    w16 = stp.tile([NB, P], BF16, tag="w16")
                    nc.vector.tensor_scalar(out=w16[:], in0=expw[:],
                                            scalar1=rec[:], scalar2=None,
                                            op0=ALU.mult)
                    # flatten [NB, P] -> [1, NB*P]: direct SBUF->SBUF gather
                    w1 = stp.tile([1, NB * P], BF16, tag="w1")
                    nc.sync.dma_start(w1[:], w16[:])
                    nc.leave_named_scope(f"sm{t}", sm_sc[0], False)

                    # (d4) block 4: h-part + U (covers the softmax latency)
                    sc_ = nc.enter_named_scope(f"d4_{t}", False)
                    for g in (4,):
                        gsl = slice(g * 512, (g + 1) * 512)
                        pg = psg_p.tile([128, 512], F32, tag="g",
                                        name=f"pg4_{g}")
                        for k in range(KH):
                            smm(pg, k, hsl(hbfT, k), w2[k][:, gsl], k < 2,
                                False)
                        umm(pg, gsl, u_t)
                        psg[g] = pg
                    nc.leave_named_scope(f"d4_{t}", sc_[0], False)

                    # (d5) block 5: h-part + U (covers the softmax too)
                    sc_ = nc.enter_named_scope(f"att{t}", False)
                    for g in (5,):
                        gsl = slice(g * 512, (g + 1) * 512)
                        pg = psg_p.tile([128, 512], F32, tag="g",
                                        name=f"pg5_{g}")
                        for k in range(KH):
                            smm(pg, k, hsl(hbfT, k), w2[k][:, gsl], k < 2,
                                False)
                        umm(pg, gsl, u_t)
                        psg[g] = pg

                    # (b) broadcast w to 128 partitions via ones-matmul;
                    # psum->sbuf bf16 copy on ACT (keeps DVE free)
                    ps_w = psw_p.tile([128, NB * P], F32, tag="w")
                    nc.tensor.matmul(ps_w[:], ones[:], w1[:],
                                     start=True, stop=True)
                    wfull = attp.tile([128, NB * P], BF16, tag="wfull")
                    nc.scalar.activation(wfull[:], ps_w[:], AF.Copy)

                    # (e) attention pooling -> attnT: product on DVE/GpSimd
                    # (split), reduce on DVE, over the packed AT tile
                    attnT = []
                    with nc.allow_low_precision("attn pooled in bf16 anyway"):
                        # chunk 0 alone first (earliest consumer), then pairs
                        groups = [(0, 1), (1, 3), (3, 5), (5, 7), (7, 8)]
                        for lo, hi in groups:
                            nk = hi - lo
                            hs = slice(lo * NB * P, hi * NB * P)
                            pr = attp.tile([128, 2 * NB * P], BF16, tag="pr")
                            prv = pr[:, 0:nk * NB * P]
                            nc.vector.tensor_tensor(
                                out=prv.rearrange("m (k x) -> m k x", k=nk),
                                in0=at_all[:, hs].rearrange(
                                    "m (k x) -> m k x", k=nk),
                                in1=bass.AP(wfull[:].tensor, wfull[:].offset,
                                            [wfull[:].ap[0], [0, nk],
                                             wfull[:].ap[1]]),
                                op=ALU.mult)
                            ab4 = abtp.tile([128, 2 * NB], BF16, tag="ab")
                            nc.vector.tensor_reduce(
                                ab4[:, 0:nk * NB],
                                prv.rearrange("m (k n p) -> m k n p", k=nk,
                                              p=P),
                                axis=AXX, op=ALU.add)
                            for kk in range(nk):
                                attnT.append(
                                    ab4[:, kk * NB:(kk + 1) * NB])
                    nc.leave_named_scope(f"att{t}", sc_[0], False)

                    # quad-stacked state for this step: blocks 4g'..4g'+3 of
                    # quad q live on partitions 32g'..32g'+31
                    GS = [blkp.tile([128, 512], F32, tag=f"GS{q}",
                                    name=f"GS{q}_{t}") for q in range(2)]
                    c_new = [blkp.tile([128, 128], F32, tag="c", bufs=4,
                                       name=f"cn{q}_{t}") for q in range(2)]
                    hbf = [blkp.tile([128, 128], BF16, tag="hbf",
                                     name=f"hbf{q}_{t}") for q in range(2)]
                    if not last:
                        hbfT_new = [htp.tile([128, 128], BF16, tag="hbfT",
                                             name=f"hT{q}_{t}")
                                    for q in range(2)]

                    def merge_block(g, pg):
                        # strip1 + strip0(+U) -> stacked row of GS[quad]
                        q, gp = divmod(g, 4)
                        row = slice(32 * gp, 32 * gp + 32)
                        g1 = blkp.tile([NB, 512], F32, tag="g1", bufs=3)
                        nc.scalar.activation(g1[:], pg[64:64 + NB, :], AF.Copy)
                        nc.vector.tensor_add(GS[q][row, :], pg[0:NB, :],
                                             g1[:])

                    def quad_math(q):
                        # all four blocks of the quad at full partition width
                        gq = GS[q]
                        sio = blkp.tile([128, 384], F32, tag="sio")
                        nc.scalar.activation(sio[:], gq[:, 0:384], AF.Sigmoid)
                        tg = blkp.tile([128, 128], F32, tag="tg")
                        nc.scalar.activation(tg[:], gq[:, 384:512], AF.Tanh)
                        m1 = blkp.tile([128, 128], F32, tag="m1")
                        nc.vector.tensor_tensor(out=m1[:], in0=sio[:, 0:128],
                                                in1=tg[:], op=ALU.mult)
                        m2 = blkp.tile([128, 128], F32, tag="m2")
                        nc.vector.tensor_tensor(out=m2[:], in0=sio[:, 128:256],
                                                in1=c_b[q][:], op=ALU.mult)
                        nc.vector.tensor_add(c_new[q][:], m1[:], m2[:])
                        tcn = blkp.tile([128, 128], F32, tag="tcn")
                        nc.scalar.activation(tcn[:], c_new[q][:], AF.Tanh)
                        nc.vector.tensor_tensor(out=hbf[q][:],
                                                in0=sio[:, 256:384],
                                                in1=tcn[:], op=ALU.mult)
                        # fp32 copy for the DRAM output
                        hf = blkp.tile([128, 128], F32, tag="hf", bufs=4,
                                       name=f"hf{q}_{t}")
                        nc.scalar.activation(hf[:], hbf[q][:], AF.Copy)
                        qsl = slice(q * 512, (q + 1) * 512)
                        nc.sync.dma_start(
                            out_d[:, t, qsl].rearrange("n (g c) -> g n c",
                                                       g=4),
                            hf[:])

                    # (f) attn-parts for blocks 0..5: first chunks k-outer so
                    # the matmuls pipeline against attnT production, then
                    # per-block finish; quad0 math after block 3's merge
                    sc_ = nc.enter_named_scope(f"f05_{t}", False)
                    for k in range(KH, KH + 4):
                        for g in range(6):
                            gsl = slice(g * 512, (g + 1) * 512)
                            smm(psg[g], k, attnT[k - KH], w2[k][:, gsl],
                                False, False)
                    for g in range(6):
                        gsl = slice(g * 512, (g + 1) * 512)
                        for k in range(KH + 4, K2):
                            smm(psg[g], k, attnT[k - KH], w2[k][:, gsl],
                                False, k >= K2 - 2)
                        merge_block(g, psg[g])
                        if g == 3:
                            quad_math(0)
                    nc.leave_named_scope(f"f05_{t}", sc_[0], False)

                    # (h) blocks 6,7: full accumulation + merges + quad1
                    sc_ = nc.enter_named_scope(f"h67_{t}", False)
                    for g in (6, 7):
                        gsl = slice(g * 512, (g + 1) * 512)
                        pg = psg_p.tile([128, 512], F32, tag="g",
                                        name=f"pg67_{g}")
                        for k in range(KH):
                            smm(pg, k, hsl(hbfT, k), w2[k][:, gsl], k < 2,
                                False)
                        umm(pg, gsl, u_t)
                        for k in range(KH, K2):
                            smm(pg, k, attnT[k - KH], w2[k][:, gsl],
                                False, k >= K2 - 2)
                        merge_block(g, pg)
                    quad_math(1)
                    nc.leave_named_scope(f"h67_{t}", sc_[0], False)

                    # (i..l) PE-transpose h back to hT layout, next
                    # step's scores first (they gate the softmax), then
                    # blocks 0..3 h-part + U
                    sc_ = nc.enter_named_scope(f"nxt{t}", False)
                    if not last:
                        psg2 = {}
                        for g in range(4):
                            psg2[g] = psg_p.tile([128, 512], F32, tag="g",
                                                 name=f"pgn{g}")
                        ps_s = psg2[0][32:64, :]
                        # PE transposes -> one shared psum bank -> sbuf
                        htq = pst_p.tile([128, 256], BF16, tag="htq",
                                         name=f"htq_{t}")
                        nc.tensor.transpose(htq[:, 0:128], hbf[0][:],
                                            eye128[:])
                        nc.vector.tensor_copy(hbfT_new[0][:], htq[:, 0:128])
                        for k in range(4):
                            nc.tensor.matmul(ps_s, hsl(hbfT_new, k), at[k],
                                             start=(k == 0), stop=False,
                                             skip_group_check=True)
                        # quad1 transpose
                        nc.tensor.transpose(htq[:, 128:256], hbf[1][:],
                                            eye128[:])
                        nc.vector.tensor_copy(hbfT_new[1][:], htq[:, 128:256])
                        for k in (4, 5, 6, 7):
                            nc.tensor.matmul(ps_s, hsl(hbfT_new, k), at[k],
                                             start=False, stop=(k == 7),
                                             skip_group_check=True)
                        for g in range(4):
                            gsl = slice(g * 512, (g + 1) * 512)
                            for k in range(KH):
                                smm(psg2[g], k, hsl(hbfT_new, k),
                                    w2[k][:, gsl], k < 2, False)
                            umm(psg2[g], gsl, u_next)
                        psg = psg2
                    nc.leave_named_scope(f"nxt{t}", sc_[0], False)

                    if not last:
                        hbfT = hbfT_new
                        c_b = c_new
                        u_t = u_next

    nc.compile()
    return nc


def prepare_inputs(x, A, Wx, Wh, Wattn, b, t_steps=T):
    """Host-side sharding + layout prep. Returns list of per-core input maps."""
    x = np.asarray(x, dtype=np.float32)
    A = np.asarray(A, dtype=np.float32)
    Wx = np.asarray(Wx, dtype=np.float32)
    Wh = np.asarray(Wh, dtype=np.float32)
    Wattn = np.asarray(Wattn, dtype=np.float32)
    b = np.asarray(b, dtype=np.float32)

    perm = _gate_perm()
    wx_p = np.ascontiguousarray(Wx[:, perm]).astype(BF)
    w2_p = np.ascontiguousarray(np.vstack([Wh, Wattn])[:, perm]).astype(BF)
    b128 = np.ascontiguousarray(
        np.broadcast_to(b[perm], (128, G))).astype(np.float32)
    mask = np.zeros((NB, NB * P), dtype=BF)
    for n in range(NB):
        mask[n, n * P:(n + 1) * P] = 1
    ones = np.ones((1, 128), dtype=BF)
    ident = np.eye(NB, dtype=BF)
    eye128 = np.eye(128, dtype=BF)

    in_maps = []
    for c in range(NCORES):
        x_c = x[c * NB:(c + 1) * NB, :t_steps]          # (NB, t, D)
        xr = x_c.transpose(1, 0, 2).reshape(t_steps * NB, D)  # t-major rows
        xT = np.ascontiguousarray(xr.T).astype(BF)       # (D, t*NB)
        A_c = A[c * NB:(c + 1) * NB].reshape(NB, H, P)
        at_c = np.ascontiguousarray(
            A_c.transpose(1, 0, 2).reshape(H, NB * P)).astype(BF)
        h0 = A_c.mean(axis=2).astype(np.float32)         # (NB, H)
        # quad-transposed initial h: tile q, col 32*g'+n, row c ->
        # h[n, (4q+g')*128 + c]
        h0Tq = np.empty((2 * 128, 128), dtype=BF)
        for k in range(8):
            q, gp = divmod(k, 4)
            h0Tq[q * 128:(q + 1) * 128, gp * 32:(gp + 1) * 32] = \
                h0[:, k * 128:(k + 1) * 128].T.astype(BF)
        # quad-stacked initial cell state: block g of quad q lives on
        # partitions 32*(g%4), columns = h dims within the block
        h0q = np.empty((2 * 128, 128), dtype=np.float32)
        for g in range(8):
            q, gp = divmod(g, 4)
            h0q[q * 128 + gp * 32:q * 128 + (gp + 1) * 32, :] = \
                h0[:, g * 128:(g + 1) * 128]
        in_maps.append({
            "xT": xT, "wx": wx_p, "w2": w2_p, "b128": b128,
            "at": at_c, "h0Tq": h0Tq, "h0q": h0q,
            "mask": mask, "ones": ones, "ident": ident,
            "eye128": eye128,
        })
    return in_maps


def kernel(x, A, Wx, Wh, Wattn, b):
    from concourse.bass_utils import run_bass_kernel_spmd

    key = T
    if key not in _NC_CACHE:
        _NC_CACHE[key] = build_nc(T)
    nc = _NC_CACHE[key]

    in_maps = prepare_inputs(x, A, Wx, Wh, Wattn, b)
    trace = bool(int(os.environ.get("KERNEL_TRACE", "0")))
    res = run_bass_kernel_spmd(nc, in_maps, core_ids=list(range(NCORES)),
                               trace=trace)
    if res.exec_time_ns is not None:
        print(f"HW exec time: {res.exec_time_ns} ns")
        kernel.last_exec_time_ns = res.exec_time_ns
    out = np.concatenate([r["out"] for r in res.results], axis=0)
    return out.astype(np.float32)


kernel.last_exec_time_ns = None


# revision 32
# speedup vs baseline: 1.0624x; 1.0153x over previous
"""Trainium2 Bass kernel for an attention-augmented LSTM (CaptioningRNN).

Reference computation (per batch n, T timesteps):
    A_flat = A.reshape(N, H, 16); h0 = c0 = A_flat.mean(-1)
    scores_t = (h_{t-1} @ A_flat) / sqrt(H); w = softmax(scores)
    attn_t = A_flat @ w
    a = x_t @ Wx + h_{t-1} @ Wh + attn_t @ Wattn + b
    i, f, o, g = split(a, 4); c_t = sig(f)*c + sig(i)*tanh(g); h_t = sig(o)*tanh(c_t)

Strategy: data-parallel over batch across 8 cores (32 batch rows each).
Per core:
  Phase A: U = x @ Wx + b precomputed for all timesteps (bf16 weights,
           rows t-major) and staged to DRAM in bf16. W2/AT/h0 for phase B
           are DMA'd concurrently on the gpsimd queue.
  Phase B: 64 recurrent steps. Gate matmul = [h; attn] (2048-dim contraction,
           bf16) against W2 = [Wh; Wattn] with gate-interleaved columns so each
           512-column block yields a full 128-dim slice of (i,f,o,g) and thus a
           128-dim slice of h/c. Attention scores are computed on the tensor
           engine (hT^T @ AT giving all batch pairs, diagonal extracted via a
           mask + strided reduce), softmax on ACT/DVE, attention pooling on
           DVE, h transposed back to hT layout with PE transposes into a
           shared PSUM bank (scores live in spare partitions 32..63 of the
           block-0 gate bank). W2/AT preload overlaps phase A's compute.

Weight-matrix column order (gate interleave): block j (512 cols) holds
original columns [i|f|o|g][j*128:(j+1)*128]. The same permutation is applied
to Wx, b and hence U.
"""

import math
import os

import numpy as np
import ml_dtypes

import concourse.bass as bass
import concourse.mybir as mybir
import concourse.tile as tile
from concourse import bacc

N, T, D, H = 256, 64, 1024, 1024
NCORES = 8
NB = N // NCORES          # 32 batch rows per core
G = 4 * H                 # 4096 gate columns
P = 16                    # attention positions (4x4)
KH = H // 128             # 8 contraction chunks for h
K2 = (2 * H) // 128       # 16 contraction chunks for [h; attn]
GB = G // 512             # 8 gate blocks of 512
F32 = mybir.dt.float32
BF16 = mybir.dt.bfloat16
BF = ml_dtypes.bfloat16

AF = mybir.ActivationFunctionType
ALU = mybir.AluOpType
AXX = mybir.AxisListType.X

_NC_CACHE = {}


def _gate_perm():
    """perm[new_col] = old_col for the gate-interleaved layout."""
    perm = np.empty(G, dtype=np.int64)
    for j in range(GB):
        for s in range(4):  # i, f, o, g
            perm[j * 512 + s * 128:(j * 512 + (s + 1) * 128)] = np.arange(
                s * H + j * 128, s * H + (j + 1) * 128)
    return perm


def build_nc(t_steps=T):
    """Build the SPMD Bass program (identical on all cores)."""
    nc = bacc.Bacc("TRN2", target_bir_lowering=False, debug=False,
                   num_devices=NCORES)

    xT_d = nc.dram_tensor("xT", [D, t_steps * NB], BF16, kind="ExternalInput")
    wx_d = nc.dram_tensor("wx", [D, G], BF16, kind="ExternalInput")
    w2_d = nc.dram_tensor("w2", [2 * H, G], BF16, kind="ExternalInput")
    b128_d = nc.dram_tensor("b128", [128, G], F32, kind="ExternalInput")
    at_d = nc.dram_tensor("at", [H, NB * P], BF16, kind="ExternalInput")
    h0Tq_d = nc.dram_tensor("h0Tq", [2 * 128, 128], BF16, kind="ExternalInput")
    h0q_d = nc.dram_tensor("h0q", [2 * 128, 128], F32, kind="ExternalInput")
    mask_d = nc.dram_tensor("mask", [NB, NB * P], BF16, kind="ExternalInput")
    ones_d = nc.dram_tensor("ones", [1, 128], BF16, kind="ExternalInput")
    ident_d = nc.dram_tensor("ident", [NB, NB], BF16, kind="ExternalInput")
    eye128_d = nc.dram_tensor("eye128", [128, 128], BF16,
                              kind="ExternalInput")
    out_d = nc.dram_tensor("out", [NB, t_steps, H], F32, kind="ExternalOutput")

    n_row_tiles = (t_steps * NB) // 128

    with tile.TileContext(nc) as tc:
        with tc.tile_pool(name="dram", bufs=1, space="DRAM") as dpool, \
             tc.tile_pool(name="res", bufs=1) as res:
            u_dram = dpool.tile([t_steps * NB, G], BF16)

            # phase-B resident tiles; DMAs issued inside phase A below so
            # the xT loads (needed first) win the HBM bandwidth race
            w2 = [res.tile([128, G], BF16, tag=f"w2_{k}", name=f"w2_{k}")
                  for k in range(K2)]
            at_all = res.tile([128, KH * NB * P], BF16, tag="at_all")
            at = [at_all[:, k * NB * P:(k + 1) * NB * P]
                  for k in range(KH)]
            mask = res.tile([NB, NB * P], BF16, tag="mask")
            ones = res.tile([1, 128], BF16, tag="ones")
            ident = res.tile([NB, NB], BF16, tag="ident")
            eye128 = res.tile([128, 128], BF16, tag="eye128")

            # ---------------- Phase A: U = x @ Wx + b ----------------
            # g-outer so only a 512-col slice of Wx/b is resident, leaving
            # room for the W2 preload above.
            with tc.tile_pool(name="pa", bufs=1) as pa, \
                 tc.tile_pool(name="pa_ps", bufs=8, space="PSUM") as pa_ps, \
                 tc.tile_pool(name="pa_sb", bufs=12) as pa_sb:
                xT = []
                qs = [nc.sync, nc.scalar, nc.gpsimd]
                for d in range(KH):
                    t_ = pa.tile([128, t_steps * NB], BF16, tag=f"xT{d}")
                    qs[d % 3].dma_start(t_[:], xT_d[d * 128:(d + 1) * 128, :])
                    xT.append(t_)
                # small phase-B residents next, then the big W2 preload last
                for k in range(KH):
                    nc.gpsimd.dma_start(
                        at_all[:, k * NB * P:(k + 1) * NB * P],
                        at_d[k * 128:(k + 1) * 128, :])
                nc.gpsimd.dma_start(mask[:], mask_d[:])
                nc.gpsimd.dma_start(ones[:], ones_d[:])
                nc.gpsimd.dma_start(ident[:], ident_d[:])
                nc.gpsimd.dma_start(eye128[:], eye128_d[:])
                for k in range(K2):
                    nc.gpsimd.dma_start(w2[k][:], w2_d[k * 128:(k + 1) * 128, :])

                for g in range(GB):
                    gs = slice(g * 512, (g + 1) * 512)
                    wxg = pa.tile([128, KH * 512], BF16, tag="wxg", bufs=2)
                    for d in range(KH):
                        nc.scalar.dma_start(
                            wxg[:, d * 512:(d + 1) * 512],
                            wx_d[d * 128:(d + 1) * 128, gs])
                    bg = pa.tile([128, 512], F32, tag="bg", bufs=2)
                    nc.scalar.dma_start(bg[:], b128_d[:, gs])
                    for m in range(n_row_tiles):
                        ms = slice(m * 128, (m + 1) * 128)
                        ps = pa_ps.tile([128, 512], F32, tag="ps")
                        for d in range(KH):
                            nc.tensor.matmul(
                                ps[:], xT[d][:, ms],
                                wxg[:, d * 512:(d + 1) * 512],
                                start=(d == 0), stop=(d == KH - 1))
                        us = pa_sb.tile([128, 512], BF16, tag="us")
                        nc.vector.tensor_add(us[:], ps[:], bg[:])
                        nc.sync.dma_start(u_dram[ms, gs], us[:])

            # ---------------- Phase B: recurrence ----------------
            with tc.tile_pool(name="ht", bufs=4) as htp, \
                 tc.tile_pool(name="u", bufs=2) as up, \
                 tc.tile_pool(name="st", bufs=2) as stp, \
                 tc.tile_pool(name="att", bufs=3) as attp, \
                 tc.tile_pool(name="abt", bufs=10) as abtp, \
                 tc.tile_pool(name="blk", bufs=2) as blkp, \
                 tc.tile_pool(name="psg", bufs=6, space="PSUM") as psg_p, \
                 tc.tile_pool(name="pst", bufs=1, space="PSUM") as pst_p, \
                 tc.tile_pool(name="psw", bufs=1, space="PSUM") as psw_p:

                hbfT = []
                for q in range(2):
                    t_ = htp.tile([128, 128], BF16, tag="hbfT",
                                  name=f"h0T_{q}")
                    nc.gpsimd.dma_start(t_[:],
                                        h0Tq_d[q * 128:(q + 1) * 128, :])
                    hbfT.append(t_)
                c_b = []
                for q in range(2):
                    t_ = blkp.tile([128, 128], F32, tag="c", bufs=4,
                                   name=f"c0_{q}")
                    nc.gpsimd.dma_start(t_[:], h0q_d[q * 128:(q + 1) * 128, :])
                    c_b.append(t_)

                def hsl(ht2, k):
                    # hT chunk k as a slice of the quad-transposed tiles
                    return ht2[k // 4][:, 32 * (k % 4):32 * (k % 4) + 32]

                u_t = up.tile([NB, G], BF16, tag="u")
                nc.gpsimd.dma_start(u_t[:], u_dram[0:NB, :])

                inv_sqrt_h = 1.0 / math.sqrt(H)

                def smm(pg, k, lhs, rhs, start, stop):
                    if isinstance(lhs, tile.Tile):
                        lhs = lhs[:]
                    j = k % 2
                    nc.tensor.matmul(pg[64 * j:64 * j + NB, :], lhs, rhs,
                                     start=start, stop=stop,
                                     tile_position=(0, 64 * j),
                                     skip_group_check=True)

                def umm(pg, gsl, u):
                    nc.tensor.matmul(pg[0:NB, :], ident[:], u[:, gsl],
                                     start=False, stop=False,
                                     tile_position=(0, 0),
                                     skip_group_check=True)

                # ---- prologue: scores S_0 (in spare partitions 96..127
                # of block 0's PSUM bank) + h-part of blocks 0..3 + U ----
                psg = {}
                for g in range(4):
                    gsl = slice(g * 512, (g + 1) * 512)
                    pg = psg_p.tile([128, 512], F32, tag="g", name=f"pg{g}")
                    if g == 0:
                        ps_s = pg[32:64, :]
                        for k in range(KH):
                            nc.tensor.matmul(ps_s, hsl(hbfT, k), at[k],
                                             start=(k == 0),
                                             stop=(k == KH - 1),
                                             skip_group_check=True)
                    for k in range(KH):
                        smm(pg, k, hsl(hbfT, k), w2[k][:, gsl], k < 2, False)
                    umm(pg, gsl, u_t)
                    psg[g] = pg

                for t in range(t_steps):
                    last = (t + 1 >= t_steps)
                    if not last:
                        u_next = up.tile([NB, G], BF16, tag="u")
                        nc.gpsimd.dma_start(
                            u_next[:], u_dram[(t + 1) * NB:(t + 2) * NB, :])

                    # (a) softmax chain for step t (scores psum -> w1)
                    sm_sc = nc.enter_named_scope(f"sm{t}", False)
                    masked = stp.tile([NB, NB * P], F32, tag="masked")
                    nc.vector.tensor_tensor(
                        out=masked[:].rearrange("m (p n) -> m p n", n=NB),
                        in0=ps_s.rearrange("m (n p) -> m p n", p=P),
                        in1=mask[:].rearrange("m (n p) -> m p n", p=P),
                        op=ALU.mult)
                    sc = stp.tile([NB, P], F32, tag="sc")
                    nc.vector.tensor_reduce(
                        sc[:], masked[:].rearrange("m (p n) -> m p n", n=NB),
                        axis=AXX, op=ALU.add)
                    # exp(x) = s/(1-s) with s = sigmoid(x): keeps the ACT
                    # table cache at {Sigmoid, Tanh} with no per-step reloads
                    sg = stp.tile([NB, P], F32, tag="sg")
                    nc.scalar.activation(sg[:], sc[:], AF.Sigmoid,
                                         scale=float(inv_sqrt_h))
                    om = stp.tile([NB, P], F32, tag="om")
                    nc.scalar.activation(om[:], sc[:], AF.Sigmoid,
                                         scale=float(-inv_sqrt_h))
                    omr = stp.tile([NB, P], F32, tag="omr")
                    nc.vector.reciprocal(omr[:], om[:])
                    expw = stp.tile([NB, P], F32, tag="expw")
                    nc.vector.tensor_tensor(out=expw[:], in0=sg[:], in1=omr[:],
                                            op=ALU.mult)
                    sume = stp.tile([NB, 1], F32, tag="sume")
                    nc.vector.tensor_reduce(sume[:], expw[:], axis=AXX,
                                            op=ALU.add)
                    rec = stp.tile([NB, 1], F32, tag="rec")
                    nc.vector.reciprocal(rec[:], sume[:])
                    w16 = stp.tile([NB, P], BF16, tag="w16")
                    nc.vector.tensor_scalar(out=w16[:], in0=expw[:],
                                            scalar1=rec[:], scalar2=None,
                                            op0=ALU.mult)
                    # flatten [NB, P] -> [1, NB*P]: direct SBUF->SBUF gather
                    w1 = stp.tile([1, NB * P], BF16, tag="w1")
                    nc.sync.dma_start(w1[:], w16[:])
                    nc.leave_named_scope(f"sm{t}", sm_sc[0], False)

                    # (d4) block 4: h-part + U (covers the softmax latency)
                    sc_ = nc.enter_named_scope(f"d4_{t}", False)
                    for g in (4,):
                        gsl = slice(g * 512, (g + 1) * 512)
                        pg = psg_p.tile([128, 512], F32, tag="g",
                                        name=f"pg4_{g}")
                        for k in range(KH):
                            smm(pg, k, hsl(hbfT, k), w2[k][:, gsl], k < 2,
                                False)
                        umm(pg, gsl, u_t)
                        psg[g] = pg
                    nc.leave_named_scope(f"d4_{t}", sc_[0], False)

                    # (d5) block 5: h-part + U (covers the softmax too)
                    sc_ = nc.enter_named_scope(f"att{t}", False)
                    for g in (5,):
                        gsl = slice(g * 512, (g + 1) * 512)
                        pg = psg_p.tile([128, 512], F32, tag="g",
                                        name=f"pg5_{g}")
                        for k in range(KH):
                            smm(pg, k, hsl(hbfT, k), w2[k][:, gsl], k < 2,
                                False)
                        umm(pg, gsl, u_t)
                        psg[g] = pg

                    # (b) broadcast w to 128 partitions via ones-matmul;
                    # psum->sbuf bf16 copy on ACT (keeps DVE free)
                    ps_w = psw_p.tile([128, NB * P], F32, tag="w")
                    nc.tensor.matmul(ps_w[:], ones[:], w1[:],
                                     start=True, stop=True)
                    wfull = attp.tile([128, NB * P], BF16, tag="wfull")
                    nc.scalar.activation(wfull[:], ps_w[:], AF.Copy)

                    # (e) attention pooling -> attnT: product on DVE/GpSimd
                    # (split), reduce on DVE, over the packed AT tile
                    attnT = []
                    with nc.allow_low_precision("attn pooled in bf16 anyway"):
                        # chunk 0 alone first (earliest consumer), then pairs
                        groups = [(0, 1), (1, 3), (3, 5), (5, 7), (7, 8)]
                        for lo, hi in groups:
                            nk = hi - lo
                            hs = slice(lo * NB * P, hi * NB * P)
                            pr = attp.tile([128, 2 * NB * P], BF16, tag="pr")
                            prv = pr[:, 0:nk * NB * P]
                            nc.vector.tensor_tensor(
                                out=prv.rearrange("m (k x) -> m k x", k=nk),
                                in0=at_all[:, hs].rearrange(
                                    "m (k x) -> m k x", k=nk),
                                in1=bass.AP(wfull[:].tensor, wfull[:].offset,
                                            [wfull[:].ap[0], [0, nk],
                                             wfull[:].ap[1]]),
                                op=ALU.mult)
                            ab4 = abtp.tile([128, 2 * NB], BF16, tag="ab")
                            nc.vector.tensor_reduce(
                                ab4[:, 0:nk * NB],
                                prv.rearrange("m (k n p) -> m k n p", k=nk,
                                              p=P),
                                axis=AXX, op=ALU.add)
                            for kk in range(nk):
                                attnT.append(
                                    ab4[:, kk * NB:(kk + 1) * NB])
                    nc.leave_named_scope(f"att{t}", sc_[0], False)

                    # quad-stacked state for this step: blocks 4g'..4g'+3 of
                    # quad q live on partitions 32g'..32g'+31
                    GS = [blkp.tile([128, 512], F32, tag=f"GS{q}",
                                    name=f"GS{q}_{t}") for q in range(2)]
                    c_new = [blkp.tile([128, 128], F32, tag="c", bufs=4,
                                       name=f"cn{q}_{t}") for q in range(2)]
                    hbf = [blkp.tile([128, 128], BF16, tag="hbf",
                                     name=f"hbf{q}_{t}") for q in range(2)]
                    if not last:
                        hbfT_new = [htp.tile([128, 128], BF16, tag="hbfT",
                                             name=f"hT{q}_{t}")
                                    for q in range(2)]

                    def merge_block(g, pg):
                        # strip1 + strip0(+U) -> stacked row of GS[quad]
                        q, gp = divmod(g, 4)
                        row = slice(32 * gp, 32 * gp + 32)
                        g1 = blkp.tile([NB, 512], F32, tag="g1", bufs=3)
                        nc.scalar.activation(g1[:], pg[64:64 + NB, :], AF.Copy)
                        nc.vector.tensor_add(GS[q][row, :], pg[0:NB, :],
                                             g1[:])

                    def quad_math(q, defer_hf=False):
                        # all four blocks of the quad at full partition width
                        gq = GS[q]
                        sio = blkp.tile([128, 384], F32, tag="sio")
                        nc.scalar.activation(sio[:], gq[:, 0:384], AF.Sigmoid)
                        tg = blkp.tile([128, 128], F32, tag="tg")
                        nc.scalar.activation(tg[:], gq[:, 384:512], AF.Tanh)
                        m1 = blkp.tile([128, 128], F32, tag="m1")
                        nc.vector.tensor_tensor(out=m1[:], in0=sio[:, 0:128],
                                                in1=tg[:], op=ALU.mult)
                        m2 = blkp.tile([128, 128], F32, tag="m2")
                        nc.vector.tensor_tensor(out=m2[:], in0=sio[:, 128:256],
                                                in1=c_b[q][:], op=ALU.mult)
                        nc.vector.tensor_add(c_new[q][:], m1[:], m2[:])
                        tcn = blkp.tile([128, 128], F32, tag="tcn")
                        nc.scalar.activation(tcn[:], c_new[q][:], AF.Tanh)
                        nc.vector.tensor_tensor(out=hbf[q][:],
                                                in0=sio[:, 256:384],
                                                in1=tcn[:], op=ALU.mult)
                        if not defer_hf:
                            emit_hf(q)

                    def emit_hf(q):
                        # fp32 copy for the DRAM output
                        hf = blkp.tile([128, 128], F32, tag="hf", bufs=4,
                                       name=f"hf{q}_{t}")
                        nc.scalar.activation(hf[:], hbf[q][:], AF.Copy)
                        qsl = slice(q * 512, (q + 1) * 512)
                        nc.sync.dma_start(
                            out_d[:, t, qsl].rearrange("n (g c) -> g n c",
                                                       g=4),
                            hf[:])

                    # (f) attn-parts for blocks 0..5: first chunks k-outer so
                    # the matmuls pipeline against attnT production, then
                    # per-block finish; quad0 math after block 3's merge
                    sc_ = nc.enter_named_scope(f"f05_{t}", False)
                    for k in range(KH, KH + 4):
                        for g in range(6):
                            gsl = slice(g * 512, (g + 1) * 512)
                            smm(psg[g], k, attnT[k - KH], w2[k][:, gsl],
                                False, False)
                    for g in range(6):
                        gsl = slice(g * 512, (g + 1) * 512)
                        for k in range(KH + 4, K2):
                            smm(psg[g], k, attnT[k - KH], w2[k][:, gsl],
                                False, k >= K2 - 2)
                        merge_block(g, psg[g])
                        if g == 3:
                            quad_math(0)
                    nc.leave_named_scope(f"f05_{t}", sc_[0], False)

                    # (h) blocks 6,7: full accumulation + merges + quad1
                    sc_ = nc.enter_named_scope(f"h67_{t}", False)
                    for g in (6, 7):
                        gsl = slice(g * 512, (g + 1) * 512)
                        pg = psg_p.tile([128, 512], F32, tag="g",
                                        name=f"pg67_{g}")
                        for k in range(KH):
                            smm(pg, k, hsl(hbfT, k), w2[k][:, gsl], k < 2,
                                False)
                        umm(pg, gsl, u_t)
                        for k in range(KH, K2):
                            smm(pg, k, attnT[k - KH], w2[k][:, gsl],
                                False, k >= K2 - 2)
                        merge_block(g, pg)
                    quad_math(1, defer_hf=not last)
                    nc.leave_named_scope(f"h67_{t}", sc_[0], False)

                    # (i..l) PE-transpose h back to hT layout, next
                    # step's scores first (they gate the softmax), then
                    # blocks 0..3 h-part + U
                    sc_ = nc.enter_named_scope(f"nxt{t}", False)
                    if not last:
                        psg2 = {}
                        for g in range(4):
                            psg2[g] = psg_p.tile([128, 512], F32, tag="g",
                                                 name=f"pgn{g}")
                        ps_s = psg2[0][32:64, :]
                        # PE transposes -> one shared psum bank -> sbuf
                        htq = pst_p.tile([128, 256], BF16, tag="htq",
                                         name=f"htq_{t}")
                        nc.tensor.transpose(htq[:, 0:128], hbf[0][:],
                                            eye128[:])
                        nc.scalar.activation(hbfT_new[0][:], htq[:, 0:128],
                                             AF.Copy)
                        for k in range(4):
                            nc.tensor.matmul(ps_s, hsl(hbfT_new, k), at[k],
                                             start=(k == 0), stop=False,
                                             skip_group_check=True)
                        # quad1 transpose
                        nc.tensor.transpose(htq[:, 128:256], hbf[1][:],
                                            eye128[:])
                        nc.scalar.activation(hbfT_new[1][:], htq[:, 128:256],
                                             AF.Copy)
                        emit_hf(1)
                        for k in (4, 5, 6, 7):
                            nc.tensor.matmul(ps_s, hsl(hbfT_new, k), at[k],
                                             start=False, stop=(k == 7),
                                             skip_group_check=True)
                        for g in range(4):
                            gsl = slice(g * 512, (g + 1) * 512)
                            for k in range(KH):
                                smm(psg2[g], k, hsl(hbfT_new, k),
                                    w2[k][:, gsl], k < 2, False)
                            umm(psg2[g], gsl, u_next)
                        psg = psg2
                    nc.leave_named_scope(f"nxt{t}", sc_[0], False)

                    if not last:
                        hbfT = hbfT_new
                        c_b = c_new
                        u_t = u_next

    nc.compile()
    return nc


def prepare_inputs(x, A, Wx, Wh, Wattn, b, t_steps=T):
    """Host-side sharding + layout prep. Returns list of per-core input maps."""
    x = np.asarray(x, dtype=np.float32)
    A = np.asarray(A, dtype=np.float32)
    Wx = np.asarray(Wx, dtype=np.float32)
    Wh = np.asarray(Wh, dtype=np.float32)
    Wattn = np.asarray(Wattn, dtype=np.float32)
    b = np.asarray(b, dtype=np.float32)

    perm = _gate_perm()
    wx_p = np.ascontiguousarray(Wx[:, perm]).astype(BF)
    w2_p = np.ascontiguousarray(np.vstack([Wh, Wattn])[:, perm]).astype(BF)
    b128 = np.ascontiguousarray(
        np.broadcast_to(b[perm], (128, G))).astype(np.float32)
    mask = np.zeros((NB, NB * P), dtype=BF)
    for n in range(NB):
        mask[n, n * P:(n + 1) * P] = 1
    ones = np.ones((1, 128), dtype=BF)
    ident = np.eye(NB, dtype=BF)
    eye128 = np.eye(128, dtype=BF)

    in_maps = []
    for c in range(NCORES):
        x_c = x[c * NB:(c + 1) * NB, :t_steps]          # (NB, t, D)
        xr = x_c.transpose(1, 0, 2).reshape(t_steps * NB, D)  # t-major rows
        xT = np.ascontiguousarray(xr.T).astype(BF)       # (D, t*NB)
        A_c = A[c * NB:(c + 1) * NB].reshape(NB, H, P)
        at_c = np.ascontiguousarray(
            A_c.transpose(1, 0, 2).reshape(H, NB * P)).astype(BF)
        h0 = A_c.mean(axis=2).astype(np.float32)         # (NB, H)
        # quad-transposed initial h: tile q, col 32*g'+n, row c ->
        # h[n, (4q+g')*128 + c]
        h0Tq = np.empty((2 * 128, 128), dtype=BF)
        for k in range(8):
            q, gp = divmod(k, 4)
            h0Tq[q * 128:(q + 1) * 128, gp * 32:(gp + 1) * 32] = \
                h0[:, k * 128:(k + 1) * 128].T.astype(BF)
        # quad-stacked initial cell state: block g of quad q lives on
        # partitions 32*(g%4), columns = h dims within the block
        h0q = np.empty((2 * 128, 128), dtype=np.float32)
        for g in range(8):
            q, gp = divmod(g, 4)
            h0q[q * 128 + gp * 32:q * 128 + (gp + 1) * 32, :] = \
                h0[:, g * 128:(g + 1) * 128]
        in_maps.append({
            "xT": xT, "wx": wx_p, "w2": w2_p, "b128": b128,
            "at": at_c, "h0Tq": h0Tq, "h0q": h0q,
            "mask": mask, "ones": ones, "ident": ident,
            "eye128": eye128,
        })
    return in_maps


def kernel(x, A, Wx, Wh, Wattn, b):
    from concourse.bass_utils import run_bass_kernel_spmd

    key = T
    if key not in _NC_CACHE:
        _NC_CACHE[key] = build_nc(T)
    nc = _NC_CACHE[key]

    in_maps = prepare_inputs(x, A, Wx, Wh, Wattn, b)
    trace = bool(int(os.environ.get("KERNEL_TRACE", "0")))
    res = run_bass_kernel_spmd(nc, in_maps, core_ids=list(range(NCORES)),
                               trace=trace)
    if res.exec_time_ns is not None:
        print(f"HW exec time: {res.exec_time_ns} ns")
        kernel.last_exec_time_ns = res.exec_time_ns
    out = np.concatenate([r["out"] for r in res.results], axis=0)
    return out.astype(np.float32)


kernel.last_exec_time_ns = None


# revision 33
# speedup vs baseline: 1.0675x; 1.0048x over previous
"""Trainium2 Bass kernel for an attention-augmented LSTM (CaptioningRNN).

Reference computation (per batch n, T timesteps):
    A_flat = A.reshape(N, H, 16); h0 = c0 = A_flat.mean(-1)
    scores_t = (h_{t-1} @ A_flat) / sqrt(H); w = softmax(scores)
    attn_t = A_flat @ w
    a = x_t @ Wx + h_{t-1} @ Wh + attn_t @ Wattn + b
    i, f, o, g = split(a, 4); c_t = sig(f)*c + sig(i)*tanh(g); h_t = sig(o)*tanh(c_t)

Strategy: data-parallel over batch across 8 cores (32 batch rows each).
Per core:
  Phase A: U = x @ Wx + b precomputed for all timesteps (bf16 weights,
           rows t-major) and staged to DRAM in bf16. W2/AT/h0 for phase B
           are DMA'd concurrently on the gpsimd queue.
  Phase B: 64 recurrent steps. Gate matmul = [h; attn] (2048-dim contraction,
           bf16) against W2 = [Wh; Wattn] with gate-interleaved columns so each
           512-column block yields a full 128-dim slice of (i,f,o,g) and thus a
           128-dim slice of h/c. Attention scores are computed on the tensor
           engine (hT^T @ AT giving all batch pairs, diagonal extracted via a
           mask + strided reduce), softmax on ACT/DVE, attention pooling on
           DVE, h transposed back to hT layout with PE transposes into a
           shared PSUM bank (scores live in spare partitions 32..63 of the
           block-0 gate bank). W2/AT preload overlaps phase A's compute.

Weight-matrix column order (gate interleave): block j (512 cols) holds
original columns [i|f|o|g][j*128:(j+1)*128]. The same permutation is applied
to Wx, b and hence U.
"""

import math
import os

import numpy as np
import ml_dtypes

import concourse.bass as bass
import concourse.mybir as mybir
import concourse.tile as tile
from concourse import bacc

N, T, D, H = 256, 64, 1024, 1024
NCORES = 8
NB = N // NCORES          # 32 batch rows per core
G = 4 * H                 # 4096 gate columns
P = 16                    # attention positions (4x4)
KH = H // 128             # 8 contraction chunks for h
K2 = (2 * H) // 128       # 16 contraction chunks for [h; attn]
GB = G // 512             # 8 gate blocks of 512
F32 = mybir.dt.float32
BF16 = mybir.dt.bfloat16
BF = ml_dtypes.bfloat16

AF = mybir.ActivationFunctionType
ALU = mybir.AluOpType
AXX = mybir.AxisListType.X

_NC_CACHE = {}


def _gate_perm():
    """perm[new_col] = old_col for the gate-interleaved layout."""
    perm = np.empty(G, dtype=np.int64)
    for j in range(GB):
        for s in range(4):  # i, f, o, g
            perm[j * 512 + s * 128:(j * 512 + (s + 1) * 128)] = np.arange(
                s * H + j * 128, s * H + (j + 1) * 128)
    return perm


def build_nc(t_steps=T):
    """Build the SPMD Bass program (identical on all cores)."""
    nc = bacc.Bacc("TRN2", target_bir_lowering=False, debug=False,
                   num_devices=NCORES)

    xT_d = nc.dram_tensor("xT", [D, t_steps * NB], BF16, kind="ExternalInput")
    wx_d = nc.dram_tensor("wx", [D, G], BF16, kind="ExternalInput")
    w2_d = nc.dram_tensor("w2", [2 * H, G], BF16, kind="ExternalInput")
    b128_d = nc.dram_tensor("b128", [128, G], F32, kind="ExternalInput")
    at_d = nc.dram_tensor("at", [H, NB * P], BF16, kind="ExternalInput")
    h0Tq_d = nc.dram_tensor("h0Tq", [2 * 128, 128], BF16, kind="ExternalInput")
    h0q_d = nc.dram_tensor("h0q", [2 * 128, 128], F32, kind="ExternalInput")
    mask_d = nc.dram_tensor("mask", [NB, NB * P], BF16, kind="ExternalInput")
    ones_d = nc.dram_tensor("ones", [1, 128], BF16, kind="ExternalInput")
    ident_d = nc.dram_tensor("ident", [NB, NB], BF16, kind="ExternalInput")
    eye128_d = nc.dram_tensor("eye128", [128, 128], BF16,
                              kind="ExternalInput")
    out_d = nc.dram_tensor("out", [NB, t_steps, H], F32, kind="ExternalOutput")

    n_row_tiles = (t_steps * NB) // 128

    with tile.TileContext(nc) as tc:
        with tc.tile_pool(name="dram", bufs=1, space="DRAM") as dpool, \
             tc.tile_pool(name="res", bufs=1) as res:
            u_dram = dpool.tile([t_steps * NB, G], BF16)

            # phase-B resident tiles; DMAs issued inside phase A below so
            # the xT loads (needed first) win the HBM bandwidth race
            w2 = [res.tile([128, G], BF16, tag=f"w2_{k}", name=f"w2_{k}")
                  for k in range(K2)]
            at_all = res.tile([128, KH * NB * P], BF16, tag="at_all")
            at = [at_all[:, k * NB * P:(k + 1) * NB * P]
                  for k in range(KH)]
            mask = res.tile([NB, NB * P], BF16, tag="mask")
            ones = res.tile([1, 128], BF16, tag="ones")
            ident = res.tile([NB, NB], BF16, tag="ident")
            eye128 = res.tile([128, 128], BF16, tag="eye128")

            # ---------------- Phase A: U = x @ Wx + b ----------------
            # g-outer so only a 512-col slice of Wx/b is resident, leaving
            # room for the W2 preload above.
            with tc.tile_pool(name="pa", bufs=1) as pa, \
                 tc.tile_pool(name="pa_ps", bufs=8, space="PSUM") as pa_ps, \
                 tc.tile_pool(name="pa_sb", bufs=12) as pa_sb:
                xT = []
                qs = [nc.sync, nc.scalar, nc.gpsimd]
                for d in range(KH):
                    t_ = pa.tile([128, t_steps * NB], BF16, tag=f"xT{d}")
                    qs[d % 3].dma_start(t_[:], xT_d[d * 128:(d + 1) * 128, :])
                    xT.append(t_)
                # small phase-B residents next, then the big W2 preload last
                for k in range(KH):
                    nc.gpsimd.dma_start(
                        at_all[:, k * NB * P:(k + 1) * NB * P],
                        at_d[k * 128:(k + 1) * 128, :])
                nc.gpsimd.dma_start(mask[:], mask_d[:])
                nc.gpsimd.dma_start(ones[:], ones_d[:])
                nc.gpsimd.dma_start(ident[:], ident_d[:])
                nc.gpsimd.dma_start(eye128[:], eye128_d[:])
                for k in range(K2):
                    nc.gpsimd.dma_start(w2[k][:], w2_d[k * 128:(k + 1) * 128, :])

                for g in range(GB):
                    gs = slice(g * 512, (g + 1) * 512)
                    wxg = pa.tile([128, KH * 512], BF16, tag="wxg", bufs=2)
                    for d in range(KH):
                        nc.scalar.dma_start(
                            wxg[:, d * 512:(d + 1) * 512],
                            wx_d[d * 128:(d + 1) * 128, gs])
                    bg = pa.tile([128, 512], F32, tag="bg", bufs=2)
                    nc.scalar.dma_start(bg[:], b128_d[:, gs])
                    for m in range(n_row_tiles):
                        ms = slice(m * 128, (m + 1) * 128)
                        ps = pa_ps.tile([128, 512], F32, tag="ps")
                        for d in range(KH):
                            nc.tensor.matmul(
                                ps[:], xT[d][:, ms],
                                wxg[:, d * 512:(d + 1) * 512],
                                start=(d == 0), stop=(d == KH - 1))
                        us = pa_sb.tile([128, 512], BF16, tag="us")
                        nc.vector.tensor_add(us[:], ps[:], bg[:])
                        nc.sync.dma_start(u_dram[ms, gs], us[:])

            # ---------------- Phase B: recurrence ----------------
            with tc.tile_pool(name="ht", bufs=4) as htp, \
                 tc.tile_pool(name="u", bufs=2) as up, \
                 tc.tile_pool(name="st", bufs=2) as stp, \
                 tc.tile_pool(name="att", bufs=3) as attp, \
                 tc.tile_pool(name="abt", bufs=10) as abtp, \
                 tc.tile_pool(name="blk", bufs=2) as blkp, \
                 tc.tile_pool(name="psg", bufs=6, space="PSUM") as psg_p, \
                 tc.tile_pool(name="pst", bufs=1, space="PSUM") as pst_p, \
                 tc.tile_pool(name="psw", bufs=1, space="PSUM") as psw_p:

                hbfT = []
                for q in range(2):
                    t_ = htp.tile([128, 128], BF16, tag="hbfT",
                                  name=f"h0T_{q}")
                    nc.gpsimd.dma_start(t_[:],
                                        h0Tq_d[q * 128:(q + 1) * 128, :])
                    hbfT.append(t_)
                c_b = []
                for q in range(2):
                    t_ = blkp.tile([128, 128], F32, tag="c", bufs=4,
                                   name=f"c0_{q}")
                    nc.gpsimd.dma_start(t_[:], h0q_d[q * 128:(q + 1) * 128, :])
                    c_b.append(t_)

                def hsl(ht2, k):
                    # hT chunk k as a slice of the quad-transposed tiles
                    return ht2[k // 4][:, 32 * (k % 4):32 * (k % 4) + 32]

                u_t = up.tile([NB, G], BF16, tag="u")
                nc.gpsimd.dma_start(u_t[:], u_dram[0:NB, :])

                inv_sqrt_h = 1.0 / math.sqrt(H)

                def smm(pg, k, lhs, rhs, start, stop):
                    if isinstance(lhs, tile.Tile):
                        lhs = lhs[:]
                    j = k % 2
                    nc.tensor.matmul(pg[64 * j:64 * j + NB, :], lhs, rhs,
                                     start=start, stop=stop,
                                     tile_position=(0, 64 * j),
                                     skip_group_check=True)

                def umm(pg, gsl, u):
                    nc.tensor.matmul(pg[0:NB, :], ident[:], u[:, gsl],
                                     start=False, stop=False,
                                     tile_position=(0, 0),
                                     skip_group_check=True)

                # ---- prologue: scores S_0 (in spare partitions 96..127
                # of block 0's PSUM bank) + h-part of blocks 0..3 + U ----
                psg = {}
                for g in range(4):
                    gsl = slice(g * 512, (g + 1) * 512)
                    pg = psg_p.tile([128, 512], F32, tag="g", name=f"pg{g}")
                    if g == 0:
                        ps_s = pg[32:64, :]
                        for k in range(KH):
                            nc.tensor.matmul(ps_s, hsl(hbfT, k), at[k],
                                             start=(k == 0),
                                             stop=(k == KH - 1),
                                             skip_group_check=True)
                    for k in range(KH):
                        smm(pg, k, hsl(hbfT, k), w2[k][:, gsl], k < 2, False)
                    umm(pg, gsl, u_t)
                    psg[g] = pg

                for t in range(t_steps):
                    last = (t + 1 >= t_steps)
                    if not last:
                        u_next = up.tile([NB, G], BF16, tag="u")
                        nc.gpsimd.dma_start(
                            u_next[:], u_dram[(t + 1) * NB:(t + 2) * NB, :])

                    # (a) softmax chain for step t (scores psum -> w1)
                    sm_sc = nc.enter_named_scope(f"sm{t}", False)
                    masked = stp.tile([NB, NB * P], F32, tag="masked")
                    nc.vector.tensor_tensor(
                        out=masked[:].rearrange("m (p n) -> m p n", n=NB),
                        in0=ps_s.rearrange("m (n p) -> m p n", p=P),
                        in1=mask[:].rearrange("m (n p) -> m p n", p=P),
                        op=ALU.mult)
                    sc = stp.tile([NB, P], F32, tag="sc")
                    nc.vector.tensor_reduce(
                        sc[:], masked[:].rearrange("m (p n) -> m p n", n=NB),
                        axis=AXX, op=ALU.add)
                    # exp(x) = s/(1-s) with s = sigmoid(x): keeps the ACT
                    # table cache at {Sigmoid, Tanh} with no per-step reloads
                    sg = stp.tile([NB, P], F32, tag="sg")
                    nc.scalar.activation(sg[:], sc[:], AF.Sigmoid,
                                         scale=float(inv_sqrt_h))
                    om = stp.tile([NB, P], F32, tag="om")
                    nc.scalar.activation(om[:], sc[:], AF.Sigmoid,
                                         scale=float(-inv_sqrt_h))
                    omr = stp.tile([NB, P], F32, tag="omr")
                    nc.vector.reciprocal(omr[:], om[:])
                    expw = stp.tile([NB, P], F32, tag="expw")
                    nc.vector.tensor_tensor(out=expw[:], in0=sg[:], in1=omr[:],
                                            op=ALU.mult)
                    sume = stp.tile([NB, 1], F32, tag="sume")
                    nc.vector.tensor_reduce(sume[:], expw[:], axis=AXX,
                                            op=ALU.add)
                    rec = stp.tile([NB, 1], F32, tag="rec")
                    nc.vector.reciprocal(rec[:], sume[:])
                    w16 = stp.tile([NB, P], BF16, tag="w16")
                    nc.vector.tensor_scalar(out=w16[:], in0=expw[:],
                                            scalar1=rec[:], scalar2=None,
                                            op0=ALU.mult)
                    # flatten [NB, P] -> [1, NB*P]: direct SBUF->SBUF gather
                    w1 = stp.tile([1, NB * P], BF16, tag="w1")
                    nc.sync.dma_start(w1[:], w16[:])
                    nc.leave_named_scope(f"sm{t}", sm_sc[0], False)

                    # (d4) block 4: h-part + U (covers the softmax latency)
                    sc_ = nc.enter_named_scope(f"d4_{t}", False)
                    for g in (4,):
                        gsl = slice(g * 512, (g + 1) * 512)
                        pg = psg_p.tile([128, 512], F32, tag="g",
                                        name=f"pg4_{g}")
                        for k in range(KH):
                            smm(pg, k, hsl(hbfT, k), w2[k][:, gsl], k < 2,
                                False)
                        umm(pg, gsl, u_t)
                        psg[g] = pg
                    nc.leave_named_scope(f"d4_{t}", sc_[0], False)

                    # (d5) block 5: h-part + U (covers the softmax too)
                    sc_ = nc.enter_named_scope(f"att{t}", False)
                    for g in (5,):
                        gsl = slice(g * 512, (g + 1) * 512)
                        pg = psg_p.tile([128, 512], F32, tag="g",
                                        name=f"pg5_{g}")
                        for k in range(KH):
                            smm(pg, k, hsl(hbfT, k), w2[k][:, gsl], k < 2,
                                False)
                        umm(pg, gsl, u_t)
                        psg[g] = pg

                    # (b) broadcast w to 128 partitions via ones-matmul;
                    # psum->sbuf bf16 copy on ACT (keeps DVE free)
                    ps_w = psw_p.tile([128, NB * P], F32, tag="w")
                    nc.tensor.matmul(ps_w[:], ones[:], w1[:],
                                     start=True, stop=True)
                    wfull = attp.tile([128, NB * P], BF16, tag="wfull")
                    nc.scalar.activation(wfull[:], ps_w[:], AF.Copy)

                    # (e) attention pooling -> attnT: product on DVE/GpSimd
                    # (split), reduce on DVE, over the packed AT tile
                    attnT = []
                    with nc.allow_low_precision("attn pooled in bf16 anyway"):
                        # chunk 0 alone first (earliest consumer), then pairs
                        groups = [(0, 1), (1, 3), (3, 5), (5, 7), (7, 8)]
                        for lo, hi in groups:
                            nk = hi - lo
                            hs = slice(lo * NB * P, hi * NB * P)
                            pr = attp.tile([128, 2 * NB * P], BF16, tag="pr")
                            prv = pr[:, 0:nk * NB * P]
                            nc.vector.tensor_tensor(
                                out=prv.rearrange("m (k x) -> m k x", k=nk),
                                in0=at_all[:, hs].rearrange(
                                    "m (k x) -> m k x", k=nk),
                                in1=bass.AP(wfull[:].tensor, wfull[:].offset,
                                            [wfull[:].ap[0], [0, nk],
                                             wfull[:].ap[1]]),
                                op=ALU.mult)
                            ab4 = abtp.tile([128, 2 * NB], BF16, tag="ab")
                            nc.vector.tensor_reduce(
                                ab4[:, 0:nk * NB],
                                prv.rearrange("m (k n p) -> m k n p", k=nk,
                                              p=P),
                                axis=AXX, op=ALU.add)
                            for kk in range(nk):
                                attnT.append(
                                    ab4[:, kk * NB:(kk + 1) * NB])
                    nc.leave_named_scope(f"att{t}", sc_[0], False)

                    # quad-stacked state for this step: blocks 4g'..4g'+3 of
                    # quad q live on partitions 32g'..32g'+31
                    GS = [blkp.tile([128, 512], F32, tag=f"GS{q}",
                                    name=f"GS{q}_{t}") for q in range(2)]
                    c_new = [blkp.tile([128, 128], F32, tag="c", bufs=4,
                                       name=f"cn{q}_{t}") for q in range(2)]
                    hbf = [blkp.tile([128, 128], BF16, tag="hbf",
                                     name=f"hbf{q}_{t}") for q in range(2)]
                    if not last:
                        hbfT_new = [htp.tile([128, 128], BF16, tag="hbfT",
                                             name=f"hT{q}_{t}")
                                    for q in range(2)]

                    def merge_block(g, pg):
                        # strip1 + strip0(+U) -> stacked row of GS[quad]
                        q, gp = divmod(g, 4)
                        row = slice(32 * gp, 32 * gp + 32)
                        g1 = blkp.tile([NB, 512], F32, tag="g1", bufs=3)
                        nc.scalar.activation(g1[:], pg[64:64 + NB, :], AF.Copy)
                        nc.vector.tensor_add(GS[q][row, :], pg[0:NB, :],
                                             g1[:])

                    def quad_math(q, defer_hf=False):
                        # all four blocks of the quad at full partition width
                        gq = GS[q]
                        sio = blkp.tile([128, 384], F32, tag="sio")
                        nc.scalar.activation(sio[:], gq[:, 0:384], AF.Sigmoid)
                        tg = blkp.tile([128, 128], F32, tag="tg")
                        nc.scalar.activation(tg[:], gq[:, 384:512], AF.Tanh)
                        m1 = blkp.tile([128, 128], F32, tag="m1")
                        nc.vector.tensor_tensor(out=m1[:], in0=sio[:, 0:128],
                                                in1=tg[:], op=ALU.mult)
                        m2 = blkp.tile([128, 128], F32, tag="m2")
                        nc.vector.tensor_tensor(out=m2[:], in0=sio[:, 128:256],
                                                in1=c_b[q][:], op=ALU.mult)
                        nc.vector.tensor_add(c_new[q][:], m1[:], m2[:])
                        tcn = blkp.tile([128, 128], F32, tag="tcn")
                        nc.scalar.activation(tcn[:], c_new[q][:], AF.Tanh)
                        nc.vector.tensor_tensor(out=hbf[q][:],
                                                in0=sio[:, 256:384],
                                                in1=tcn[:], op=ALU.mult)
                        if not defer_hf:
                            emit_hf(q)

                    def emit_hf(q):
                        # fp32 copy for the DRAM output
                        hf = blkp.tile([128, 128], F32, tag="hf", bufs=4,
                                       name=f"hf{q}_{t}")
                        nc.scalar.activation(hf[:], hbf[q][:], AF.Copy)
                        qsl = slice(q * 512, (q + 1) * 512)
                        nc.sync.dma_start(
                            out_d[:, t, qsl].rearrange("n (g c) -> g n c",
                                                       g=4),
                            hf[:])

                    # (f) attn-parts for blocks 0..5: first chunks k-outer so
                    # the matmuls pipeline against attnT production, then
                    # per-block finish; quad0 math after block 3's merge
                    sc_ = nc.enter_named_scope(f"f05_{t}", False)
                    for k in range(KH, KH + 4):
                        for g in range(6):
                            gsl = slice(g * 512, (g + 1) * 512)
                            smm(psg[g], k, attnT[k - KH], w2[k][:, gsl],
                                False, False)
                    for g in range(6):
                        gsl = slice(g * 512, (g + 1) * 512)
                        for k in range(KH + 4, K2):
                            smm(psg[g], k, attnT[k - KH], w2[k][:, gsl],
                                False, k >= K2 - 2)
                        merge_block(g, psg[g])
                        if g == 3:
                            quad_math(0, defer_hf=not last)
                    nc.leave_named_scope(f"f05_{t}", sc_[0], False)

                    # (h) blocks 6,7: full accumulation + merges + quad1
                    sc_ = nc.enter_named_scope(f"h67_{t}", False)
                    for g in (6, 7):
                        gsl = slice(g * 512, (g + 1) * 512)
                        pg = psg_p.tile([128, 512], F32, tag="g",
                                        name=f"pg67_{g}")
                        for k in range(KH):
                            smm(pg, k, hsl(hbfT, k), w2[k][:, gsl], k < 2,
                                False)
                        umm(pg, gsl, u_t)
                        for k in range(KH, K2):
                            smm(pg, k, attnT[k - KH], w2[k][:, gsl],
                                False, k >= K2 - 2)
                        merge_block(g, pg)
                    quad_math(1, defer_hf=not last)
                    nc.leave_named_scope(f"h67_{t}", sc_[0], False)

                    # (i..l) PE-transpose h back to hT layout, next
                    # step's scores first (they gate the softmax), then
                    # blocks 0..3 h-part + U
                    sc_ = nc.enter_named_scope(f"nxt{t}", False)
                    if not last:
                        psg2 = {}
                        for g in range(4):
                            psg2[g] = psg_p.tile([128, 512], F32, tag="g",
                                                 name=f"pgn{g}")
                        ps_s = psg2[0][32:64, :]
                        # PE transposes -> one shared psum bank -> sbuf
                        htq = pst_p.tile([128, 256], BF16, tag="htq",
                                         name=f"htq_{t}")
                        nc.tensor.transpose(htq[:, 0:128], hbf[0][:],
                                            eye128[:])
                        nc.scalar.activation(hbfT_new[0][:], htq[:, 0:128],
                                             AF.Copy)
                        for k in range(4):
                            nc.tensor.matmul(ps_s, hsl(hbfT_new, k), at[k],
                                             start=(k == 0), stop=False,
                                             skip_group_check=True)
                        # quad1 transpose
                        nc.tensor.transpose(htq[:, 128:256], hbf[1][:],
                                            eye128[:])
                        nc.scalar.activation(hbfT_new[1][:], htq[:, 128:256],
                                             AF.Copy)
                        emit_hf(1)
                        emit_hf(0)
                        for k in (4, 5, 6, 7):
                            nc.tensor.matmul(ps_s, hsl(hbfT_new, k), at[k],
                                             start=False, stop=(k == 7),
                                             skip_group_check=True)
                        for g in range(4):
                            gsl = slice(g * 512, (g + 1) * 512)
                            for k in range(KH):
                                smm(psg2[g], k, hsl(hbfT_new, k),
                                    w2[k][:, gsl], k < 2, False)
                            umm(psg2[g], gsl, u_next)
                        psg = psg2
                    nc.leave_named_scope(f"nxt{t}", sc_[0], False)

                    if not last:
                        hbfT = hbfT_new
                        c_b = c_new
                        u_t = u_next

    nc.compile()
    return nc


def prepare_inputs(x, A, Wx, Wh, Wattn, b, t_steps=T):
    """Host-side sharding + layout prep. Returns list of per-core input maps."""
    x = np.asarray(x, dtype=np.float32)
    A = np.asarray(A, dtype=np.float32)
    Wx = np.asarray(Wx, dtype=np.float32)
    Wh = np.asarray(Wh, dtype=np.float32)
    Wattn = np.asarray(Wattn, dtype=np.float32)
    b = np.asarray(b, dtype=np.float32)

    perm = _gate_perm()
    wx_p = np.ascontiguousarray(Wx[:, perm]).astype(BF)
    w2_p = np.ascontiguousarray(np.vstack([Wh, Wattn])[:, perm]).astype(BF)
    b128 = np.ascontiguousarray(
        np.broadcast_to(b[perm], (128, G))).astype(np.float32)
    mask = np.zeros((NB, NB * P), dtype=BF)
    for n in range(NB):
        mask[n, n * P:(n + 1) * P] = 1
    ones = np.ones((1, 128), dtype=BF)
    ident = np.eye(NB, dtype=BF)
    eye128 = np.eye(128, dtype=BF)

    in_maps = []
    for c in range(NCORES):
        x_c = x[c * NB:(c + 1) * NB, :t_steps]          # (NB, t, D)
        xr = x_c.transpose(1, 0, 2).reshape(t_steps * NB, D)  # t-major rows
        xT = np.ascontiguousarray(xr.T).astype(BF)       # (D, t*NB)
        A_c = A[c * NB:(c + 1) * NB].reshape(NB, H, P)
        at_c = np.ascontiguousarray(
            A_c.transpose(1, 0, 2).reshape(H, NB * P)).astype(BF)
        h0 = A_c.mean(axis=2).astype(np.float32)         # (NB, H)
        # quad-transposed initial h: tile q, col 32*g'+n, row c ->
        # h[n, (4q+g')*128 + c]
        h0Tq = np.empty((2 * 128, 128), dtype=BF)
        for k in range(8):
            q, gp = divmod(k, 4)
            h0Tq[q * 128:(q + 1) * 128, gp * 32:(gp + 1) * 32] = \
                h0[:, k * 128:(k + 1) * 128].T.astype(BF)
        # quad-stacked initial cell state: block g of quad q lives on
        # partitions 32*(g%4), columns = h dims within the block
        h0q = np.empty((2 * 128, 128), dtype=np.float32)
        for g in range(8):
            q, gp = divmod(g, 4)
            h0q[q * 128 + gp * 32:q * 128 + (gp + 1) * 32, :] = \
                h0[:, g * 128:(g + 1) * 128]
        in_maps.append({
            "xT": xT, "wx": wx_p, "w2": w2_p, "b128": b128,
            "at": at_c, "h0Tq": h0Tq, "h0q": h0q,
            "mask": mask, "ones": ones, "ident": ident,
            "eye128": eye128,
        })
    return in_maps


def kernel(x, A, Wx, Wh, Wattn, b):
    from concourse.bass_utils import run_bass_kernel_spmd

    key = T
    if key not in _NC_CACHE:
        _NC_CACHE[key] = build_nc(T)
    nc = _NC_CACHE[key]

    in_maps = prepare_inputs(x, A, Wx, Wh, Wattn, b)
    trace = bool(int(os.environ.get("KERNEL_TRACE", "0")))
    res = run_bass_kernel_spmd(nc, in_maps, core_ids=list(range(NCORES)),
                               trace=trace)
    if res.exec_time_ns is not None:
        print(f"HW exec time: {res.exec_time_ns} ns")
        kernel.last_exec_time_ns = res.exec_time_ns
    out = np.concatenate([r["out"] for r in res.results], axis=0)
    return out.astype(np.float32)


kernel.last_exec_time_ns = None
